# revision 1
# baseline (speedup 1.0000x reference)
"""Trainium2 Bass kernel for GTLayer (graph-transformer layer), 8-core SPMD.

Math (matching the torch-style reference exactly):
  QH = h @ Wq.T + bq ; KH, VH likewise                          [N, F]
  per head hh (raw reshape): q_hh = QH[hh*512:(hh+1)*512].view(N, 32)
  t = q @ k.T * scale ; P = softmax(t * A, axis=-1) ; O = P @ v
  y = concat-heads @ Wo.T + bo
  x = BN1(y + h); out = BN2(x + relu(x@W1.T+c1)@W2.T+c2)

Distribution: a row permutation m~ = s*512+u  <->  m = u*8+s turns every
head-view block into natural-layout slices: head (hh) with score block
(s, s') uses QH[hh-rows, s-cols] x KH[hh-rows, s'-cols].  Device d owns
score rows m~ in [d*512, (d+1)*512) (i.e. Q feature-slice d), computes
S^T tiles (scores transposed: partition = m~', free = m~), multiplies by
the host-permuted A^T block, exponentiates (scale folded into exp), and
accumulates O^T = [V|1]^T-style augmented matmul giving the softmax
denominators for free in row 32 of PSUM.  K^T and V are computed
redundantly on every device (cheaper than an all-gather).  An AllToAll
re-shards from feature-slices to row-blocks for Wo/BN/FFN, which run in
transposed layout (feature on partitions) so BatchNorm stats are
per-partition sums reduced with a tiny AllReduce.
"""

import sys

sys.path.insert(0, "/opt/trn_rl_repo")

from contextlib import ExitStack

import numpy as np

import concourse.bacc as bacc
import concourse.bass as bass
import concourse.tile as tile
from concourse import mybir
from concourse.bass_utils import run_bass_kernel_spmd

ND = 8          # devices
N = 4096        # nodes
F = 256         # hidden
H = 8           # heads
DH = 32         # head dim
L = N // ND     # 512 rows per device
F2 = 2 * F      # ffn hidden
SCALE = DH ** -0.5
EPS = 1e-5
f32 = mybir.dt.float32

# vecs packing (per-partition scalar columns, [128, NVEC])
VEC_BQ = 0        # bq slice d        (32 rows used)
VEC_BK = 1        # bk halves         (2 cols)
VEC_BO = 3        # bo halves         (2 cols)
VEC_C1 = 5        # c1 quarters       (4 cols)
VEC_C2 = 9        # c2 halves         (2 cols)
VEC_G1 = 11       # g1 halves         (2)
VEC_BE1 = 13      # be1 halves        (2)
VEC_G2 = 15       # g2 halves         (2)
VEC_BE2 = 17      # be2 halves        (2)
NVEC = 19

_CACHE = {}


def _build(bv_zero: bool):
    nc = bacc.Bacc("TRN2", target_bir_lowering=False, debug=False,
                   num_devices=ND)

    hT_d = nc.dram_tensor("hT", [F, N], f32, kind="ExternalInput").ap()
    atp_d = nc.dram_tensor("atp", [N, L], f32, kind="ExternalInput").ap()
    wqT_d = nc.dram_tensor("wqT", [F, DH], f32, kind="ExternalInput").ap()
    wkT_d = nc.dram_tensor("wkT", [F, F], f32, kind="ExternalInput").ap()
    wvT_d = nc.dram_tensor("wvT", [F, F], f32, kind="ExternalInput").ap()
    woT_d = nc.dram_tensor("woT", [F, F], f32, kind="ExternalInput").ap()
    w1T_d = nc.dram_tensor("w1T", [F, F2], f32, kind="ExternalInput").ap()
    w2T_d = nc.dram_tensor("w2T", [F2, F], f32, kind="ExternalInput").ap()
    vecs_d = nc.dram_tensor("vecs", [128, NVEC], f32, kind="ExternalInput").ap()
    h1T_d = nc.dram_tensor("h1T", [F, L], f32, kind="ExternalInput").ap()
    if not bv_zero:
        bvrow_d = nc.dram_tensor("bvrow", [1, F], f32,
                                 kind="ExternalInput").ap()
    out_d = nc.dram_tensor("out", [F, L], f32, kind="ExternalOutput").ap()

    # collective staging (DRAM only)
    ot_dram = nc.dram_tensor("ot_stage", [H * DH, L], f32)
    ya_dram = nc.dram_tensor("ya_stage", [H * DH, L], f32)
    rs_dram = nc.dram_tensor("rs_stage", [H, 512], f32)
    st1_in = nc.dram_tensor("st1_in", [128, 4], f32)
    st1_out = nc.dram_tensor("st1_out", [128, 4], f32, addr_space="Shared")
    st2_in = nc.dram_tensor("st2_in", [128, 4], f32)
    st2_out = nc.dram_tensor("st2_out", [128, 4], f32, addr_space="Shared")

    groups = [list(range(ND))]

    with tile.TileContext(nc) as tc, ExitStack() as ctx:
        big = ctx.enter_context(tc.tile_pool(name="big", bufs=4))
        res = ctx.enter_context(tc.tile_pool(name="res", bufs=1))
        ps = ctx.enter_context(tc.tile_pool(name="ps", bufs=3, space="PSUM"))
        po = ctx.enter_context(tc.tile_pool(name="po", bufs=2, space="PSUM"))
        pt_pool = ctx.enter_context(tc.tile_pool(name="ptp", bufs=2))
        et_pool = ctx.enter_context(tc.tile_pool(name="etp", bufs=2))
        small = ctx.enter_context(tc.tile_pool(name="small", bufs=2))
        ffn = ctx.enter_context(tc.tile_pool(name="ffn", bufs=1))

        # ---- resident tensors ----
        kt_sb = res.tile([128, 2, N], f32)        # K^T: [f%128, f//128, n]
        v_sb = res.tile([128, 32, H, DH + 1], f32)  # V nat + ones col
        qt4 = res.tile([128, N], f32)             # Q^T slice, 4x replicated
        vecs = res.tile([128, NVEC], f32)
        wq_sb = res.tile([128, 2 * DH], f32)
        wk_sb = res.tile([128, 2 * F], f32)
        wv_sb = res.tile([128, 2 * F], f32)
        wo_sb = res.tile([128, 2 * F], f32)
        w1_sb = res.tile([128, 2 * F2], f32)
        w2_sb = res.tile([128, 4 * F], f32)
        h1_sb = res.tile([128, 2, L], f32)        # h^T[:, d-block] residual

        nc.sync.dma_start(out=vecs, in_=vecs_d)
        nc.vector.memset(v_sb[:, :, :, DH:DH + 1], 1.0)
        for gc in range(2):
            nc.sync.dma_start(out=wq_sb[:, gc * DH:(gc + 1) * DH],
                              in_=wqT_d[gc * 128:(gc + 1) * 128, :])
            nc.sync.dma_start(out=wk_sb[:, gc * F:(gc + 1) * F],
                              in_=wkT_d[gc * 128:(gc + 1) * 128, :])
            nc.sync.dma_start(out=wv_sb[:, gc * F:(gc + 1) * F],
                              in_=wvT_d[gc * 128:(gc + 1) * 128, :])
            nc.sync.dma_start(out=wo_sb[:, gc * F:(gc + 1) * F],
                              in_=woT_d[gc * 128:(gc + 1) * 128, :])
            nc.sync.dma_start(out=w1_sb[:, gc * F2:(gc + 1) * F2],
                              in_=w1T_d[gc * 128:(gc + 1) * 128, :])
        for kc in range(4):
            nc.sync.dma_start(out=w2_sb[:, kc * F:(kc + 1) * F],
                              in_=w2T_d[kc * 128:(kc + 1) * 128, :])
        if not bv_zero:
            bvb = res.tile([128, F], f32)
            nc.sync.dma_start(out=bvb, in_=bvrow_d.to_broadcast([128, F]))

        # ---- h^T (2 chunks) in shared "big" slots, freed for A^T tiles ----
        ht = [big.tile([128, N], f32, tag="big", name=f"ht{i}") for i in range(2)]
        for gc in range(2):
            nc.sync.dma_start(out=ht[gc], in_=hT_d[gc * 128:(gc + 1) * 128, :])
            nc.sync.dma_start(out=h1_sb[:, gc, :],
                              in_=h1T_d[gc * 128:(gc + 1) * 128, :])

        # ---- projections ----
        # Q^T slice d: [32, N] -> replicated to 4 partition bands
        for nck in range(8):
            pq = ps.tile([128, 1024], f32, tag="ps")
            for gc in range(2):
                nc.tensor.matmul(pq[0:DH, 0:512],
                                 lhsT=wq_sb[:, gc * DH:(gc + 1) * DH],
                                 rhs=ht[gc][:, nck * 512:(nck + 1) * 512],
                                 start=(gc == 0), stop=(gc == 1))
            nc.vector.tensor_scalar_add(qt4[0:DH, nck * 512:(nck + 1) * 512],
                                        pq[0:DH, 0:512],
                                        vecs[0:DH, VEC_BQ:VEC_BQ + 1])
        for band in range(1, 4):
            nc.sync.dma_start(out=qt4[band * DH:(band + 1) * DH, :],
                              in_=qt4[0:DH, :])

        # K^T full: [256, N] as [128, 2, N]
        for hf in range(2):
            for ncs in range(4):
                pk = ps.tile([128, 1024], f32, tag="ps")
                for half in range(2):
                    for gc in range(2):
                        nc.tensor.matmul(
                            pk[:, half * 512:(half + 1) * 512],
                            lhsT=wk_sb[:, gc * F + hf * 128:
                                       gc * F + (hf + 1) * 128],
                            rhs=ht[gc][:, (ncs * 2 + half) * 512:
                                       (ncs * 2 + half + 1) * 512],
                            start=(gc == 0), stop=(gc == 1))
                nc.vector.tensor_scalar_add(
                    kt_sb[:, hf, ncs * 1024:(ncs + 1) * 1024], pk,
                    vecs[:, VEC_BK + hf:VEC_BK + hf + 1])

        # V natural: [N, 256] as 32 n-tiles; strided into v_sb (+ones col)
        for nt in range(32):
            pv = ps.tile([128, 1024], f32, tag="ps")
            for gc in range(2):
                nc.tensor.matmul(pv[:, 0:F],
                                 lhsT=ht[gc][:, nt * 128:(nt + 1) * 128],
                                 rhs=wv_sb[:, gc * F:(gc + 1) * F],
                                 start=(gc == 0), stop=(gc == 1))
            src = pv[:, 0:F].rearrange("p (s c) -> p s c", c=DH)
            if bv_zero:
                nc.scalar.activation(v_sb[:, nt, :, 0:DH], src,
                                     mybir.ActivationFunctionType.Copy)
            else:
                nc.vector.tensor_add(
                    v_sb[:, nt, :, 0:DH], src,
                    bvb.rearrange("p (s c) -> p s c", c=DH))

        # ---- A^T tiles (shared pool with ht; band-major order) ----
        at_t = [big.tile([128, 8, 512], f32, tag="big", name=f"at{i}") for i in range(4)]
        for j in range(32):
            hfj, upj, bj = j // 16, (j % 16) // 4, j % 4
            p_tile = hfj * 16 + bj * 4 + upj
            nc.sync.dma_start(
                out=at_t[j // 8][:, j % 8, :],
                in_=atp_d[p_tile * 128:(p_tile + 1) * 128, :])

        # ---- attention ----
        for hh in range(H):
            pso = po.tile([128, 512], f32, tag="po")
            first = True
            for g in range(8):          # (hf, u') groups
                hf, up = g // 4, g % 4
                for pair in range(2):   # bands (0,1) then (2,3)
                    psp = ps.tile([128, 1024], f32, tag="ps")
                    for bi in range(2):
                        band = pair * 2 + bi
                        sp = hf * 4 + band
                        nc.tensor.matmul(
                            psp[:, bi * 512:(bi + 1) * 512],
                            lhsT=kt_sb[band * DH:(band + 1) * DH, hf,
                                       hh * 512 + up * 128:
                                       hh * 512 + (up + 1) * 128],
                            rhs=qt4[band * DH:(band + 1) * DH,
                                    hh * 512:(hh + 1) * 512],
                            start=True, stop=True,
                            tile_position=(band * DH, 0))
                    j0 = g * 4 + pair * 2
                    pt = pt_pool.tile([128, 2, 512], f32, tag="pt")
                    nc.vector.tensor_mul(
                        pt, psp.rearrange("p (b c) -> p b c", b=2),
                        at_t[j0 // 8][:, j0 % 8:j0 % 8 + 2, :])
                    et = et_pool.tile([128, 2, 512], f32, tag="et")
                    nc.scalar.activation(et, pt,
                                         mybir.ActivationFunctionType.Exp,
                                         scale=SCALE)
                    for bi in range(2):
                        band = pair * 2 + bi
                        sp = hf * 4 + band
                        nt = hh * 4 + up
                        nc.tensor.matmul(
                            pso[0:DH + 1, :],
                            lhsT=v_sb[:, nt, sp, :],
                            rhs=et[:, bi, :],
                            start=first, stop=(g == 7 and pair == 1 and bi == 1))
                        first = False
            rsi = small.tile([1, 512], f32, tag="rsi", bufs=1)
            nc.vector.reciprocal(rsi, pso[DH:DH + 1, :])
            nc.sync.dma_start(out=rs_dram.ap()[hh:hh + 1, :], in_=rsi)
            rb = small.tile([DH, 512], f32, tag="rb", bufs=1)
            nc.sync.dma_start(
                out=rb, in_=rs_dram.ap()[hh:hh + 1, :].to_broadcast([DH, 512]))
            on = small.tile([DH, 512], f32, tag="on", bufs=1)
            nc.vector.tensor_mul(on, pso[0:DH, :], rb)
            nc.sync.dma_start(out=ot_dram.ap()[hh * DH:(hh + 1) * DH, :],
                              in_=on)

        # ---- exchange to row-blocks ----
        nc.gpsimd.collective_compute(
            "AllToAll", mybir.AluOpType.bypass, replica_groups=groups,
            ins=[ot_dram.ap()], outs=[ya_dram.ap()])

        yt = [ffn.tile([128, L], f32, tag=f"yt{i}", name=f"yt{i}") for i in range(2)]
        for gc in range(2):
            nc.sync.dma_start(out=yt[gc],
                              in_=ya_dram.ap()[gc * 128:(gc + 1) * 128, :])

        # ---- Wo + residual -> x1 ; BN1 stats ----
        x1 = [ffn.tile([128, L], f32, tag=f"x1{i}", name=f"x1{i}") for i in range(2)]
        stat_in = ffn.tile([128, 4], f32, tag="stat")
        for fo in range(2):
            py = ps.tile([128, 1024], f32, tag="ps")
            for gc in range(2):
                nc.tensor.matmul(
                    py[:, 0:512],
                    lhsT=wo_sb[:, gc * F + fo * 128: gc * F + (fo + 1) * 128],
                    rhs=yt[gc],
                    start=(gc == 0), stop=(gc == 1))
            nc.vector.tensor_scalar_add(py[:, 0:512], py[:, 0:512],
                                        vecs[:, VEC_BO + fo:VEC_BO + fo + 1])
            nc.vector.tensor_add(x1[fo], py[:, 0:512], h1_sb[:, fo, :])
            nc.vector.tensor_reduce(stat_in[:, fo:fo + 1], x1[fo],
                                    axis=mybir.AxisListType.X,
                                    op=mybir.AluOpType.add)
            sq = small.tile([128, 512], f32, tag="sq", bufs=1)
            nc.scalar.activation(sq, x1[fo],
                                 mybir.ActivationFunctionType.Square,
                                 accum_out=stat_in[:, 2 + fo:3 + fo])
        nc.sync.dma_start(out=st1_in.ap(), in_=stat_in)
        nc.gpsimd.collective_compute(
            "AllReduce", mybir.AluOpType.add, replica_groups=groups,
            ins=[st1_in.ap()], outs=[st1_out.ap()])
        st1 = ffn.tile([128, 4], f32, tag="st1")
        nc.sync.dma_start(out=st1, in_=st1_out.ap())

        def bn_affine(st, vg, vbe):
            """per-half affine coeffs a,b from [sumx(2), sumx2(2)] cols."""
            a_list, b_list = [], []
            for hfi in range(2):
                mu = small.tile([128, 1], f32, tag="mu", bufs=1)
                nc.vector.tensor_scalar_mul(mu, st[:, hfi:hfi + 1], 1.0 / N)
                ex2 = small.tile([128, 1], f32, tag="ex2", bufs=1)
                nc.vector.tensor_scalar_mul(ex2, st[:, 2 + hfi:3 + hfi],
                                            1.0 / N)
                var = small.tile([128, 1], f32, tag="var", bufs=1)
                nc.vector.tensor_mul(var, mu, mu)
                nc.vector.tensor_sub(var, ex2, var)
                nc.vector.tensor_scalar_add(var, var, EPS)
                sd = small.tile([128, 1], f32, tag="sd", bufs=1)
                nc.scalar.sqrt(sd, var)
                rv = small.tile([128, 1], f32, tag="rv", bufs=1)
                nc.vector.reciprocal(rv, sd)
                a = small.tile([128, 1], f32, tag=f"a{hfi}", bufs=1)
                nc.vector.tensor_mul(a, vecs[:, vg + hfi:vg + hfi + 1], rv)
                b = small.tile([128, 1], f32, tag=f"b{hfi}", bufs=1)
                nc.vector.tensor_mul(b, mu, a)
                nc.vector.tensor_sub(b, vecs[:, vbe + hfi:vbe + hfi + 1], b)
                a_list.append(a)
                b_list.append(b)
            return a_list, b_list

        a1, b1 = bn_affine(st1, VEC_G1, VEC_BE1)
        x2 = [ffn.tile([128, L], f32, tag=f"x2{i}", name=f"x2{i}") for i in range(2)]
        for hfi in range(2):
            nc.vector.tensor_scalar(x2[hfi], x1[hfi], a1[hfi], b1[hfi],
                                    op0=mybir.AluOpType.mult,
                                    op1=mybir.AluOpType.add)

        # ---- FFN ----
        za = [ffn.tile([128, L], f32, tag=("yt%d" % i if i < 2 else "x1%d" % (i - 2)), name=f"za{i}") for i in range(4)]
        for f2t in range(4):
            pz = ps.tile([128, 1024], f32, tag="ps")
            for gc in range(2):
                nc.tensor.matmul(
                    pz[:, 0:512],
                    lhsT=w1_sb[:, gc * F2 + f2t * 128: gc * F2 + (f2t + 1) * 128],
                    rhs=x2[gc],
                    start=(gc == 0), stop=(gc == 1))
            nc.scalar.activation(za[f2t], pz[:, 0:512],
                                 mybir.ActivationFunctionType.Relu,
                                 bias=vecs[:, VEC_C1 + f2t:VEC_C1 + f2t + 1])

        x3 = [ffn.tile([128, L], f32, tag=f"yt{i}", name=f"x3{i}") for i in range(2)]
        stat2 = ffn.tile([128, 4], f32, tag="stat2")
        for fo in range(2):
            p2 = ps.tile([128, 1024], f32, tag="ps")
            for kc in range(4):
                nc.tensor.matmul(
                    p2[:, 0:512],
                    lhsT=w2_sb[:, kc * F + fo * 128: kc * F + (fo + 1) * 128],
                    rhs=za[kc],
                    start=(kc == 0), stop=(kc == 3))
            nc.vector.tensor_scalar_add(p2[:, 0:512], p2[:, 0:512],
                                        vecs[:, VEC_C2 + fo:VEC_C2 + fo + 1])
            nc.vector.tensor_add(x3[fo], p2[:, 0:512], x2[fo])
            nc.vector.tensor_reduce(stat2[:, fo:fo + 1], x3[fo],
                                    axis=mybir.AxisListType.X,
                                    op=mybir.AluOpType.add)
            sq2 = small.tile([128, 512], f32, tag="sq", bufs=1)
            nc.scalar.activation(sq2, x3[fo],
                                 mybir.ActivationFunctionType.Square,
                                 accum_out=stat2[:, 2 + fo:3 + fo])
        nc.sync.dma_start(out=st2_in.ap(), in_=stat2)
        nc.gpsimd.collective_compute(
            "AllReduce", mybir.AluOpType.add, replica_groups=groups,
            ins=[st2_in.ap()], outs=[st2_out.ap()])
        st2 = ffn.tile([128, 4], f32, tag="st2")
        nc.sync.dma_start(out=st2, in_=st2_out.ap())

        a2, b2 = bn_affine(st2, VEC_G2, VEC_BE2)
        for hfi in range(2):
            xo = small.tile([128, 512], f32, tag="xo", bufs=1)
            nc.vector.tensor_scalar(xo, x3[hfi], a2[hfi], b2[hfi],
                                    op0=mybir.AluOpType.mult,
                                    op1=mybir.AluOpType.add)
            nc.sync.dma_start(out=out_d[hfi * 128:(hfi + 1) * 128, :], in_=xo)

    nc.compile()
    return nc


def _get_nc(bv_zero):
    key = bv_zero
    if key not in _CACHE:
        _CACHE[key] = _build(bv_zero)
    return _CACHE[key]


def kernel(A, h, Wq, bq, Wk, bk, Wv, bv, Wo, bo, W1, c1, W2, c2,
           g1, be1, g2, be2):
    A = np.asarray(A, np.float32)
    h = np.asarray(h, np.float32)

    idx = np.arange(N)
    perm = (idx % L) * H + idx // L        # m~ -> m
    Ap = A[np.ix_(perm, perm)]
    ApT = np.ascontiguousarray(Ap.T)       # [m~', m~]
    hT = np.ascontiguousarray(h.T)

    wqT = np.ascontiguousarray(np.asarray(Wq, np.float32).T)
    wkT = np.ascontiguousarray(np.asarray(Wk, np.float32).T)
    wvT = np.ascontiguousarray(np.asarray(Wv, np.float32).T)
    woT = np.ascontiguousarray(np.asarray(Wo, np.float32).T)
    w1T = np.ascontiguousarray(np.asarray(W1, np.float32).T)
    w2T = np.ascontiguousarray(np.asarray(W2, np.float32).T)

    bv_zero = not np.any(np.asarray(bv))
    nc = _get_nc(bv_zero)

    def halves(v):
        return np.asarray(v, np.float32).reshape(2, 128).T  # [128, 2]

    in_maps = []
    for d in range(ND):
        vecs = np.zeros((128, NVEC), np.float32)
        vecs[0:DH, VEC_BQ] = np.asarray(bq, np.float32)[d * DH:(d + 1) * DH]
        vecs[:, VEC_BK:VEC_BK + 2] = halves(bk)
        vecs[:, VEC_BO:VEC_BO + 2] = halves(bo)
        vecs[:, VEC_C1:VEC_C1 + 4] = np.asarray(c1, np.float32).reshape(4, 128).T
        vecs[:, VEC_C2:VEC_C2 + 2] = halves(c2)
        vecs[:, VEC_G1:VEC_G1 + 2] = halves(g1)
        vecs[:, VEC_BE1:VEC_BE1 + 2] = halves(be1)
        vecs[:, VEC_G2:VEC_G2 + 2] = halves(g2)
        vecs[:, VEC_BE2:VEC_BE2 + 2] = halves(be2)
        m = {
            "hT": hT,
            "atp": np.ascontiguousarray(ApT[:, d * L:(d + 1) * L]),
            "wqT": np.ascontiguousarray(wqT[:, d * DH:(d + 1) * DH]),
            "wkT": wkT, "wvT": wvT, "woT": woT, "w1T": w1T, "w2T": w2T,
            "vecs": vecs,
            "h1T": np.ascontiguousarray(hT[:, d * L:(d + 1) * L]),
        }
        if not bv_zero:
            m["bvrow"] = np.asarray(bv, np.float32).reshape(1, F)
        in_maps.append(m)

    res = run_bass_kernel_spmd(nc, in_maps, core_ids=list(range(ND)))
    out = np.concatenate(
        [np.asarray(r["out"]).T for r in res.results], axis=0)
    return out.astype(np.float32)


if __name__ == "__main__":
    pass



# revision 18
# speedup vs baseline: 2.3921x; 2.3921x over previous
"""Trainium2 Bass kernel for GTLayer (graph-transformer layer), 8-core SPMD.

Math (matching the torch-style reference exactly):
  QH = h @ Wq.T + bq ; KH, VH likewise                          [N, F]
  per head hh (raw reshape): q_hh = QH[hh*512:(hh+1)*512].view(N, 32)
  t = q @ k.T * scale ; P = softmax(t * A, axis=-1) ; O = P @ v
  y = concat-heads @ Wo.T + bo
  x = BN1(y + h); out = BN2(x + relu(x@W1.T+c1)@W2.T+c2)

Distribution: a row permutation m~ = s*512+u  <->  m = u*8+s turns every
head-view block into natural-layout slices (see v1 notes).  Device d owns
score rows m~ in [d*512, (d+1)*512): it computes S^T tiles (partition =
key m~', free = query m~), applies the multiplicative adjacency mask and
the exponential in a single fused custom DVE op (Schraudolph exp2: the
int16 value a*x+b IS the fp16 bit pattern of exp(scale*x)), then
accumulates O^T plus softmax denominators via an augmented [V | 1]
matmul, col-packed 2x by query halves.  All matmuls run in bf16/fp16
(fp32 PSUM).  An AllToAll (bf16) re-shards head-blocks to row-blocks for
Wo/BN/FFN, which run transposed (features on partitions) so BatchNorm
stats are per-partition sums reduced with a tiny AllReduce.
"""

import math
import os
import sys

sys.path.insert(0, "/opt/trn_rl_repo")

from contextlib import ExitStack

import numpy as np

import concourse.bacc as bacc
import concourse.bass as bass
import concourse.tile as tile
from concourse import mybir
from concourse.bass_utils import run_bass_kernel_spmd

ND = 8          # devices
N = 4096        # nodes
F = 256         # hidden
H = 8           # heads
DH = 32         # head dim
L = N // ND     # 512 rows per device
F2 = 2 * F      # ffn hidden
SCALE = DH ** -0.5
EPS = 1e-5
f32 = mybir.dt.float32
bf16 = mybir.dt.bfloat16
fp16 = mybir.dt.float16
i16 = mybir.dt.int16

# Schraudolph constants targeting the fp16 bit pattern:
#   int16 z = A_E * (t*A) + B_E  ==  fp16-bits of ~exp(SCALE * t*A)
A_E = 1024.0 * math.log2(math.e) * SCALE
B_E = float(15 << 10)

# vecs packing for the generic (nonzero-bias) path [128, NVEC]
VEC_BQ = 0        # bq slice d        (32 rows used)
VEC_BK = 1        # bk halves         (2 cols)
VEC_BO = 3        # bo halves         (2 cols)
VEC_C1 = 5        # c1 quarters       (4 cols)
VEC_C2 = 9        # c2 halves         (2 cols)
VEC_G1 = 11       # g1 halves         (2)
VEC_BE1 = 13      # be1 halves        (2)
VEC_G2 = 15       # g2 halves         (2)
VEC_BE2 = 17      # be2 halves        (2)
NVEC = 19
# V bias (varies along the free axis) ships as a broadcast row instead.

_CACHE = {}
_FUSED_OP = None


def _get_fused_op():
    """Register (once) the fused mask-multiply + Schraudolph-exp DVE op:
    out = (in0 * in1) * s0 + s1, written as int16 (the fp16 bit pattern)."""
    global _FUSED_OP
    if _FUSED_OP is not None:
        return _FUSED_OP
    import concourse.dve_ops as dvo
    from concourse.dve_spec import Spec, Src0, Src1, C0, C1, lower
    from concourse.dve_spec import _has_src1
    from concourse.dve_uop import DveOpSpec

    name = "TT_AFFINE_I16_ANT"
    for op in dvo.OPS:
        if op.name == name:
            _FUSED_OP = op
            return op

    spec = Spec(
        body=Src0 * Src1 * C0 + C1,
        reference=lambda in0, in1, s0, s1, imm2: (
            in0.astype(np.float32) * in1.astype(np.float32) * s0 + s1
        ),
    )
    row = dvo._CUSTOM_DVE_ROW_BASE + len(dvo.OPS)
    assert row < 0x20, "no free custom-DVE opcode rows"
    shas = {}
    for ver in ("v3", "v4"):
        tmp = DveOpSpec(
            name=name, opcode=row, uops=lower(spec, ver=ver),
            rd1_en=_has_src1(spec),
        )
        shas[ver] = tmp.sha(ver)
    op = dvo.DveOp(name, spec, subdim=False, uops_sha=shas)
    dvo.OPS.append(op)
    dvo.CUSTOM_DVE_SPECS[name] = spec
    dvo._SUB_OPCODE_FOR_NAME[name] = row
    _FUSED_OP = op
    return op


def _build(generic: bool, exp_mode: str):
    nc = bacc.Bacc("TRN2", target_bir_lowering=False, debug=False,
                   num_devices=ND)

    hT_d = nc.dram_tensor("hT", [F, N], bf16, kind="ExternalInput").ap()
    atp_d = nc.dram_tensor("atp", [N, L], bf16, kind="ExternalInput").ap()
    wqT_d = nc.dram_tensor("wqT", [F, DH], bf16, kind="ExternalInput").ap()
    wkT_d = nc.dram_tensor("wkT", [F, F], bf16, kind="ExternalInput").ap()
    wvT_d = nc.dram_tensor("wvT", [F, F], bf16, kind="ExternalInput").ap()
    woT_d = nc.dram_tensor("woT", [F, F], bf16, kind="ExternalInput").ap()
    w1T_d = nc.dram_tensor("w1T", [F, F2], bf16, kind="ExternalInput").ap()
    w2T_d = nc.dram_tensor("w2T", [F2, F], bf16, kind="ExternalInput").ap()
    h1T_d = nc.dram_tensor("h1T", [F, L], bf16, kind="ExternalInput").ap()
    if generic:
        vecs_d = nc.dram_tensor("vecs", [128, NVEC], f32,
                                kind="ExternalInput").ap()
        bvrow_d = nc.dram_tensor("bvrow", [1, F], f32,
                                 kind="ExternalInput").ap()
    out_d = nc.dram_tensor("out", [F, L], f32, kind="ExternalOutput").ap()

    # collective staging (DRAM only)
    dbg = os.environ.get("BASS_GT_DEBUG") == "1"
    ot_dram = nc.dram_tensor("ot_stage", [H * DH, L], bf16)
    ya_dram = nc.dram_tensor("ya_stage", [H * DH, L], bf16)
    if dbg:
        ot_dump = nc.dram_tensor("ot_dump", [H * DH, L], bf16,
                                 kind="ExternalOutput").ap()
        ya_dump = nc.dram_tensor("ya_dump", [H * DH, L], bf16,
                                 kind="ExternalOutput").ap()
        ps_dump = nc.dram_tensor("ps_dump", [128, 1024], f32,
                                 kind="ExternalOutput").ap()
        et_dump = nc.dram_tensor("et_dump", [128, 1024], i16,
                                 kind="ExternalOutput").ap()
        pso_dump = nc.dram_tensor("pso_dump", [128, 256], f32,
                                  kind="ExternalOutput").ap()
        r_dump = nc.dram_tensor("r_dump", [64, 256], f32,
                                kind="ExternalOutput").ap()
        v4_dump = nc.dram_tensor("v4_dump", [128, 64], fp16,
                                 kind="ExternalOutput").ap()
    st1_in = nc.dram_tensor("st1_in", [128, 4], f32)
    st1_out = nc.dram_tensor("st1_out", [128, 4], f32, addr_space="Shared")
    st2_in = nc.dram_tensor("st2_in", [128, 4], f32)
    st2_out = nc.dram_tensor("st2_out", [128, 4], f32, addr_space="Shared")

    groups = [list(range(ND))]
    use_fused = exp_mode == "custom"
    if use_fused:
        fused_op = _get_fused_op()

    with tile.TileContext(nc) as tc, ExitStack() as ctx:
        res = ctx.enter_context(tc.tile_pool(name="res", bufs=1))
        psp = ctx.enter_context(tc.tile_pool(name="psp", bufs=2, space="PSUM"))
        pso = ctx.enter_context(tc.tile_pool(name="pso", bufs=2, space="PSUM"))
        pj = ctx.enter_context(tc.tile_pool(name="pj", bufs=2, space="PSUM"))
        et_pool = ctx.enter_context(tc.tile_pool(name="etp", bufs=2))
        pt_pool = ctx.enter_context(tc.tile_pool(name="ptp", bufs=2))
        small = ctx.enter_context(tc.tile_pool(name="small", bufs=2))
        ffn = ctx.enter_context(tc.tile_pool(name="ffn", bufs=1))

        # ---- resident tensors ----
        ht = res.tile([128, 2, N], bf16)          # h^T: [f%128, f//128, n]
        at_sb = res.tile([128, 32, 512], bf16)    # A^T tiles, custom order
        kt = res.tile([128, 2, N], bf16)          # K^T
        qt4 = res.tile([128, N], bf16)            # Q^T slice, 4x replicated
        v4 = res.tile([128, 32, 8, 64], fp16)     # [V | ones] per (nt, sp)
        h1_sb = res.tile([128, 2, L], bf16)       # h^T[:, d-block] residual
        wq_sb = res.tile([128, 2, DH], bf16)
        wk_sb = res.tile([128, 2, F], bf16)
        wv_sb = res.tile([128, 2, F], bf16)
        wo_sb = res.tile([128, 2, F], bf16)
        w1_sb = res.tile([128, 2, F2], bf16)
        w2_sb = res.tile([128, 4, F], bf16)
        if generic:
            vecs = res.tile([128, NVEC], f32)
            nc.sync.dma_start(out=vecs, in_=vecs_d)
            bvb = res.tile([128, F], f32)
            nc.sync.dma_start(out=bvb, in_=bvrow_d.to_broadcast([128, F]))

        for gc in range(2):
            nc.sync.dma_start(out=ht[:, gc, :],
                              in_=hT_d[gc * 128:(gc + 1) * 128, :])
        nc.sync.dma_start(out=wq_sb[:, 0, :], in_=wqT_d[0:128, :])
        nc.sync.dma_start(out=wq_sb[:, 1, :], in_=wqT_d[128:256, :])
        for gc in range(2):
            nc.sync.dma_start(out=wk_sb[:, gc, :],
                              in_=wkT_d[gc * 128:(gc + 1) * 128, :])
            nc.sync.dma_start(out=wv_sb[:, gc, :],
                              in_=wvT_d[gc * 128:(gc + 1) * 128, :])
            nc.sync.dma_start(out=wo_sb[:, gc, :],
                              in_=woT_d[gc * 128:(gc + 1) * 128, :])
            nc.sync.dma_start(out=w1_sb[:, gc, :],
                              in_=w1T_d[gc * 128:(gc + 1) * 128, :])
            nc.sync.dma_start(out=h1_sb[:, gc, :],
                              in_=h1T_d[gc * 128:(gc + 1) * 128, :])
        for kc in range(4):
            nc.sync.dma_start(out=w2_sb[:, kc, :],
                              in_=w2T_d[kc * 128:(kc + 1) * 128, :])
        # A^T tiles: storage idx = ((hf*2+pair)*4+up)*2+bi covers source
        # row-chunk (s2*4+up), s2 = hf*4+pair*2+bi
        for hf in range(2):
            for pair in range(2):
                for up in range(4):
                    for bi in range(2):
                        idx = ((hf * 2 + pair) * 4 + up) * 2 + bi
                        s2 = hf * 4 + pair * 2 + bi
                        src = s2 * 4 + up
                        nc.sync.dma_start(
                            out=at_sb[:, idx, :],
                            in_=atp_d[src * 128:(src + 1) * 128, :])
        nc.vector.memset(v4[:, :, :, DH:2 * DH], 1.0)

        # ---- projections, head-sliced for a pipelined start ----
        for hh in range(H):
            cs = slice(hh * 512, (hh + 1) * 512)
            # Q^T chunk -> qt4[0:32, cs], then replicate to bands 1..3
            pq = pj.tile([128, 512], f32, tag="pj")
            for gc in range(2):
                nc.tensor.matmul(pq[0:DH, :], lhsT=wq_sb[:, gc, :],
                                 rhs=ht[:, gc, cs],
                                 start=(gc == 0), stop=(gc == 1))
            if generic:
                nc.vector.tensor_scalar_add(
                    qt4[0:DH, cs], pq[0:DH, :],
                    vecs[0:DH, VEC_BQ:VEC_BQ + 1])
            else:
                nc.scalar.copy(qt4[0:DH, cs], pq[0:DH, :])
            for band in range(1, 4):
                nc.sync.dma_start(out=qt4[band * DH:(band + 1) * DH, cs],
                                  in_=qt4[0:DH, cs])
            # K^T chunks
            for hf in range(2):
                pk = pj.tile([128, 512], f32, tag="pj")
                for gc in range(2):
                    nc.tensor.matmul(
                        pk, lhsT=wk_sb[:, gc, hf * 128:(hf + 1) * 128],
                        rhs=ht[:, gc, cs],
                        start=(gc == 0), stop=(gc == 1))
                if generic:
                    nc.vector.tensor_scalar_add(
                        kt[:, hf, cs], pk, vecs[:, VEC_BK + hf:VEC_BK + hf + 1])
                else:
                    nc.scalar.copy(kt[:, hf, cs], pk)
            # V natural tiles (with room for the ones block)
            for up in range(4):
                nt = hh * 4 + up
                pv = pj.tile([128, 512], f32, tag="pj")
                for gc in range(2):
                    nc.tensor.matmul(pv[:, 0:F],
                                     lhsT=ht[:, gc, nt * 128:(nt + 1) * 128],
                                     rhs=wv_sb[:, gc, :],
                                     start=(gc == 0), stop=(gc == 1))
                src = pv[:, 0:F].rearrange("p (s c) -> p s c", c=DH)
                if generic:
                    nc.vector.tensor_add(
                        v4[:, nt, :, 0:DH], src,
                        bvb.rearrange("p (s c) -> p s c", c=DH))
                else:
                    nc.scalar.copy(v4[:, nt, :, 0:DH], src)

        # ---- attention ----
        for hh in range(H):
            pso_t = pso.tile([128, 256], f32, tag="pso")
            first = [True, True]          # per m1-half accumulation chain
            n_grp = 0
            for hf in range(2):
                for up in range(4):
                    for pair in range(2):
                        n_grp += 1
                        g2 = (hf * 2 + pair) * 4 + up
                        psp_t = psp.tile([128, 2, 512], f32, tag="psp")
                        for bi in range(2):
                            band = pair * 2 + bi
                            nc.tensor.matmul(
                                psp_t[:, bi, :],
                                lhsT=kt[band * DH:(band + 1) * DH, hf,
                                        hh * 512 + up * 128:
                                        hh * 512 + (up + 1) * 128],
                                rhs=qt4[band * DH:(band + 1) * DH,
                                        hh * 512:(hh + 1) * 512],
                                start=True, stop=True,
                                tile_position=(band * DH, 0))
                        at_ap = at_sb[:, g2 * 2:g2 * 2 + 2, :]
                        is_dbg_tile = dbg and hh == 0 and hf == 0 \
                            and up == 0 and pair == 0
                        if is_dbg_tile:
                            ps_scr = small.tile([128, 1024], f32,
                                                tag="psscr", bufs=1)
                            nc.scalar.copy(
                                ps_scr, psp_t.rearrange("p a b -> p (a b)"))
                            nc.sync.dma_start(out=ps_dump, in_=ps_scr)
                        if use_fused:
                            et_t = et_pool.tile([128, 2, 512], i16, tag="et")
                            nc.vector._custom_dve(
                                fused_op, out=et_t, in0=psp_t, in1=at_ap,
                                s0=A_E, s1=B_E)
                        else:
                            pt_t = pt_pool.tile([128, 2, 512], bf16, tag="pt")
                            nc.vector.tensor_mul(pt_t, psp_t, at_ap)
                            et_t = et_pool.tile([128, 2, 512], fp16, tag="et")
                            nc.scalar.activation(
                                et_t, pt_t, mybir.ActivationFunctionType.Exp,
                                scale=SCALE)
                        if is_dbg_tile:
                            nc.sync.dma_start(
                                out=et_dump,
                                in_=et_t.rearrange("p a b -> p (a b)"))
                            nc.sync.dma_start(out=v4_dump,
                                              in_=v4[:, 0, 0, :])
                        last_grp = n_grp == 16
                        for bi in range(2):
                            sp = hf * 4 + pair * 2 + bi
                            nt = hh * 4 + up
                            for mh in range(2):
                                rhs = et_t[:, bi, mh * 256:(mh + 1) * 256]
                                if use_fused:
                                    rhs = rhs.bitcast(fp16)
                                nc.tensor.matmul(
                                    pso_t[mh * 64:(mh + 1) * 64, :],
                                    lhsT=v4[:, nt, sp, :],
                                    rhs=rhs,
                                    start=first[mh],
                                    stop=(last_grp and bi == 1),
                                    tile_position=(0, mh * 64))
                                first[mh] = False
            # normalize: rows [mh*64+32, mh*64+64) hold the denominators
            if dbg and hh == 0:
                pso_scr = small.tile([128, 256], f32, tag="psoscr", bufs=1)
                nc.scalar.copy(pso_scr, pso_t)
                nc.sync.dma_start(out=pso_dump, in_=pso_scr)
            ot_sb = small.tile([DH, 512], bf16, tag="ot")
            for mh in range(2):
                den_sb = small.tile([DH, 256], f32, tag="den", bufs=2)
                nc.scalar.copy(den_sb,
                               pso_t[mh * 64 + DH:mh * 64 + 2 * DH, :])
                r = small.tile([DH, 256], f32, tag="r", bufs=2)
                nc.vector.reciprocal_approx_fast(r, den_sb)
                if dbg and hh == 0:
                    nc.sync.dma_start(
                        out=r_dump[mh * DH:(mh + 1) * DH, :], in_=r)
                nc.vector.tensor_mul(
                    ot_sb[:, mh * 256:(mh + 1) * 256],
                    pso_t[mh * 64:mh * 64 + DH, :], r)
            nc.sync.dma_start(out=ot_dram.ap()[hh * DH:(hh + 1) * DH, :],
                              in_=ot_sb)
            if dbg:
                nc.sync.dma_start(out=ot_dump[hh * DH:(hh + 1) * DH, :],
                                  in_=ot_sb)

        # ---- exchange to row-blocks ----
        nc.gpsimd.collective_compute(
            "AllToAll", mybir.AluOpType.bypass, replica_groups=groups,
            ins=[ot_dram.ap()], outs=[ya_dram.ap()])

        yt = ffn.tile([128, 2, L], bf16, tag="yt")
        for gc in range(2):
            nc.sync.dma_start(out=yt[:, gc, :],
                              in_=ya_dram.ap()[gc * 128:(gc + 1) * 128, :])
            if dbg:
                nc.sync.dma_start(out=ya_dump[gc * 128:(gc + 1) * 128, :],
                                  in_=yt[:, gc, :])

        # ---- Wo + residual -> x1 ; BN1 stats ----
        x1 = ffn.tile([128, 2, L], bf16, tag="x1")
        stat_in = ffn.tile([128, 4], f32, tag="stat")
        sq_scr = ffn.tile([128, L], bf16, tag="sq")
        for fo in range(2):
            py = pj.tile([128, 512], f32, tag="pj")
            for gc in range(2):
                nc.tensor.matmul(
                    py,
                    lhsT=wo_sb[:, gc, fo * 128:(fo + 1) * 128],
                    rhs=yt[:, gc, :],
                    start=(gc == 0), stop=(gc == 1))
            if generic:
                nc.vector.tensor_scalar_add(py, py,
                                            vecs[:, VEC_BO + fo:VEC_BO + fo + 1])
            nc.vector.tensor_add(x1[:, fo, :], py, h1_sb[:, fo, :])
            nc.vector.tensor_reduce(stat_in[:, fo:fo + 1], x1[:, fo, :],
                                    axis=mybir.AxisListType.X,
                                    op=mybir.AluOpType.add)
            nc.scalar.activation(sq_scr, x1[:, fo, :],
                                 mybir.ActivationFunctionType.Square,
                                 accum_out=stat_in[:, 2 + fo:3 + fo])
        nc.sync.dma_start(out=st1_in.ap(), in_=stat_in)
        nc.gpsimd.collective_compute(
            "AllReduce", mybir.AluOpType.add, replica_groups=groups,
            ins=[st1_in.ap()], outs=[st1_out.ap()])
        st1 = ffn.tile([128, 4], f32, tag="st1")
        nc.sync.dma_start(out=st1, in_=st1_out.ap())

        def bn_affine(st, vg, vbe):
            """per-half affine (a, b): out = x*a - b."""
            a_list, b_list = [], []
            for hfi in range(2):
                mu = small.tile([128, 1], f32, tag="mu", bufs=1)
                nc.vector.tensor_scalar_mul(mu, st[:, hfi:hfi + 1], 1.0 / N)
                ex2 = small.tile([128, 1], f32, tag="ex2", bufs=1)
                nc.vector.tensor_scalar_mul(ex2, st[:, 2 + hfi:3 + hfi],
                                            1.0 / N)
                var = small.tile([128, 1], f32, tag="var", bufs=1)
                nc.vector.tensor_mul(var, mu, mu)
                nc.vector.tensor_sub(var, ex2, var)
                nc.vector.tensor_scalar_add(var, var, EPS)
                sd = small.tile([128, 1], f32, tag="sd", bufs=1)
                nc.scalar.sqrt(sd, var)
                rv = small.tile([128, 1], f32, tag="rv", bufs=1)
                nc.vector.reciprocal(rv, sd)
                if generic:
                    a = small.tile([128, 1], f32, tag=f"a{hfi}", bufs=1)
                    nc.vector.tensor_mul(a, vecs[:, vg + hfi:vg + hfi + 1], rv)
                    b = small.tile([128, 1], f32, tag=f"b{hfi}", bufs=1)
                    nc.vector.tensor_mul(b, mu, a)
                    nc.vector.tensor_sub(b, b,
                                         vecs[:, vbe + hfi:vbe + hfi + 1])
                else:
                    a = rv
                    b = small.tile([128, 1], f32, tag=f"b{hfi}", bufs=1)
                    nc.vector.tensor_mul(b, mu, a)
                a_list.append(a)
                b_list.append(b)
            return a_list, b_list

        a1, b1 = bn_affine(st1, VEC_G1, VEC_BE1)
        x2 = ffn.tile([128, 2, L], bf16, tag="x2")
        for hfi in range(2):
            nc.vector.tensor_scalar(x2[:, hfi, :], x1[:, hfi, :],
                                    a1[hfi], b1[hfi],
                                    op0=mybir.AluOpType.mult,
                                    op1=mybir.AluOpType.subtract)

        # ---- FFN ----
        za = ffn.tile([128, 4, L], bf16, tag="za")
        for f2t in range(4):
            pz = pj.tile([128, 512], f32, tag="pj")
            for gc in range(2):
                nc.tensor.matmul(
                    pz,
                    lhsT=w1_sb[:, gc, f2t * 128:(f2t + 1) * 128],
                    rhs=x2[:, gc, :],
                    start=(gc == 0), stop=(gc == 1))
            if generic:
                nc.scalar.activation(za[:, f2t, :], pz,
                                     mybir.ActivationFunctionType.Relu,
                                     bias=vecs[:, VEC_C1 + f2t:VEC_C1 + f2t + 1])
            else:
                nc.scalar.activation(za[:, f2t, :], pz,
                                     mybir.ActivationFunctionType.Relu)

        x3 = ffn.tile([128, 2, L], bf16, tag="x3")
        stat2 = ffn.tile([128, 4], f32, tag="stat2")
        for fo in range(2):
            p2 = pj.tile([128, 512], f32, tag="pj")
            for kc in range(4):
                nc.tensor.matmul(
                    p2,
                    lhsT=w2_sb[:, kc, fo * 128:(fo + 1) * 128],
                    rhs=za[:, kc, :],
                    start=(kc == 0), stop=(kc == 3))
            if generic:
                nc.vector.tensor_scalar_add(p2, p2,
                                            vecs[:, VEC_C2 + fo:VEC_C2 + fo + 1])
            nc.vector.tensor_add(x3[:, fo, :], p2, x2[:, fo, :])
            nc.vector.tensor_reduce(stat2[:, fo:fo + 1], x3[:, fo, :],
                                    axis=mybir.AxisListType.X,
                                    op=mybir.AluOpType.add)
            nc.scalar.activation(sq_scr, x3[:, fo, :],
                                 mybir.ActivationFunctionType.Square,
                                 accum_out=stat2[:, 2 + fo:3 + fo])
        nc.sync.dma_start(out=st2_in.ap(), in_=stat2)
        nc.gpsimd.collective_compute(
            "AllReduce", mybir.AluOpType.add, replica_groups=groups,
            ins=[st2_in.ap()], outs=[st2_out.ap()])
        st2 = ffn.tile([128, 4], f32, tag="st2")
        nc.sync.dma_start(out=st2, in_=st2_out.ap())

        a2, b2 = bn_affine(st2, VEC_G2, VEC_BE2)
        for hfi in range(2):
            xo = small.tile([128, 512], f32, tag="xo", bufs=2)
            nc.vector.tensor_scalar(xo, x3[:, hfi, :], a2[hfi], b2[hfi],
                                    op0=mybir.AluOpType.mult,
                                    op1=mybir.AluOpType.subtract)
            nc.sync.dma_start(out=out_d[hfi * 128:(hfi + 1) * 128, :], in_=xo)

    nc.compile()
    return nc


def _get_nc(generic, exp_mode):
    key = (generic, exp_mode)
    if key not in _CACHE:
        _CACHE[key] = _build(generic, exp_mode)
    return _CACHE[key]


def kernel(A, h, Wq, bq, Wk, bk, Wv, bv, Wo, bo, W1, c1, W2, c2,
           g1, be1, g2, be2):
    import ml_dtypes
    nbf = ml_dtypes.bfloat16

    A = np.asarray(A, np.float32)
    h = np.asarray(h, np.float32)

    idx = np.arange(N)
    perm = (idx % L) * H + idx // L        # m~ -> m
    Ap = A[np.ix_(perm, perm)]
    ApT = np.ascontiguousarray(Ap.T).astype(nbf)   # [m~', m~]
    hT = np.ascontiguousarray(h.T)
    hTb = hT.astype(nbf)

    def tb(x):
        return np.ascontiguousarray(np.asarray(x, np.float32).T).astype(nbf)

    wqT = tb(Wq)
    wkT = tb(Wk)
    wvT = tb(Wv)
    woT = tb(Wo)
    w1T = tb(W1)
    w2T = tb(W2)

    generic = any(
        np.any(np.asarray(v))
        for v in (bq, bk, bv, bo, c1, c2, be1, be2)
    ) or np.any(np.asarray(g1) != 1) or np.any(np.asarray(g2) != 1)
    exp_mode = os.environ.get("BASS_GT_EXPMODE", "custom")
    nc = _get_nc(generic, exp_mode)

    def halves(v):
        return np.asarray(v, np.float32).reshape(2, 128).T  # [128, 2]

    in_maps = []
    for d in range(ND):
        m = {
            "hT": hTb,
            "atp": np.ascontiguousarray(ApT[:, d * L:(d + 1) * L]),
            "wqT": np.ascontiguousarray(wqT[:, d * DH:(d + 1) * DH]),
            "wkT": wkT, "wvT": wvT, "woT": woT, "w1T": w1T, "w2T": w2T,
            "h1T": np.ascontiguousarray(hTb[:, d * L:(d + 1) * L]),
        }
        if generic:
            vecs = np.zeros((128, NVEC), np.float32)
            vecs[0:DH, VEC_BQ] = np.asarray(bq, np.float32)[d * DH:(d + 1) * DH]
            vecs[:, VEC_BK:VEC_BK + 2] = halves(bk)
            vecs[:, VEC_BO:VEC_BO + 2] = halves(bo)
            vecs[:, VEC_C1:VEC_C1 + 4] = np.asarray(c1, np.float32).reshape(4, 128).T
            vecs[:, VEC_C2:VEC_C2 + 2] = halves(c2)
            vecs[:, VEC_G1:VEC_G1 + 2] = halves(g1)
            vecs[:, VEC_BE1:VEC_BE1 + 2] = halves(be1)
            vecs[:, VEC_G2:VEC_G2 + 2] = halves(g2)
            vecs[:, VEC_BE2:VEC_BE2 + 2] = halves(be2)
            m["vecs"] = vecs
            m["bvrow"] = np.asarray(bv, np.float32).reshape(1, F)
        in_maps.append(m)

    res = run_bass_kernel_spmd(nc, in_maps, core_ids=list(range(ND)))
    out = np.concatenate(
        [np.asarray(r["out"]).T for r in res.results], axis=0)
    return out.astype(np.float32)


if __name__ == "__main__":
    pass


# revision 23
# speedup vs baseline: 2.4571x; 1.0272x over previous
"""Trainium2 Bass kernel for GTLayer (graph-transformer layer), 8-core SPMD.

Math (matching the torch-style reference exactly):
  QH = h @ Wq.T + bq ; KH, VH likewise                          [N, F]
  per head hh (raw reshape): q_hh = QH[hh*512:(hh+1)*512].view(N, 32)
  t = q @ k.T * scale ; P = softmax(t * A, axis=-1) ; O = P @ v
  y = concat-heads @ Wo.T + bo
  x = BN1(y + h); out = BN2(x + relu(x@W1.T+c1)@W2.T+c2)

Distribution: a row permutation m~ = s*512+u  <->  m = u*8+s turns every
head-view block into natural-layout slices (see v1 notes).  Device d owns
score rows m~ in [d*512, (d+1)*512): it computes S^T tiles (partition =
key m~', free = query m~), applies the multiplicative adjacency mask and
the exponential in a single fused custom DVE op (Schraudolph exp2: the
int16 value a*x+b IS the fp16 bit pattern of exp(scale*x)), then
accumulates O^T plus softmax denominators via an augmented [V | 1]
matmul, col-packed 2x by query halves.  All matmuls run in bf16/fp16
(fp32 PSUM).  An AllToAll (bf16) re-shards head-blocks to row-blocks for
Wo/BN/FFN, which run transposed (features on partitions) so BatchNorm
stats are per-partition sums reduced with a tiny AllReduce.
"""

import math
import os
import sys

sys.path.insert(0, "/opt/trn_rl_repo")

from contextlib import ExitStack

import numpy as np

import concourse.bacc as bacc
import concourse.bass as bass
import concourse.tile as tile
from concourse import mybir
from concourse.bass_utils import run_bass_kernel_spmd

ND = 8          # devices
N = 4096        # nodes
F = 256         # hidden
H = 8           # heads
DH = 32         # head dim
L = N // ND     # 512 rows per device
F2 = 2 * F      # ffn hidden
SCALE = DH ** -0.5
EPS = 1e-5
f32 = mybir.dt.float32
bf16 = mybir.dt.bfloat16
fp16 = mybir.dt.float16
i16 = mybir.dt.int16

# Schraudolph constants targeting the fp16 bit pattern:
#   int16 z = A_E * (t*A) + B_E  ==  fp16-bits of ~exp(SCALE * t*A)
A_E = 1024.0 * math.log2(math.e) * SCALE
B_E = float(15 << 10)

# vecs packing for the generic (nonzero-bias) path [128, NVEC]
VEC_BQ = 0        # bq slice d        (32 rows used)
VEC_BK = 1        # bk halves         (2 cols)
VEC_BO = 3        # bo halves         (2 cols)
VEC_C1 = 5        # c1 quarters       (4 cols)
VEC_C2 = 9        # c2 halves         (2 cols)
VEC_G1 = 11       # g1 halves         (2)
VEC_BE1 = 13      # be1 halves        (2)
VEC_G2 = 15       # g2 halves         (2)
VEC_BE2 = 17      # be2 halves        (2)
NVEC = 19
# V bias (varies along the free axis) ships as a broadcast row instead.

_CACHE = {}
_FUSED_OP = None


def _get_fused_op():
    """Register (once) the fused mask-multiply + Schraudolph-exp DVE op:
    out = (in0 * in1) * s0 + s1, written as int16 (the fp16 bit pattern)."""
    global _FUSED_OP
    if _FUSED_OP is not None:
        return _FUSED_OP
    import concourse.dve_ops as dvo
    from concourse.dve_spec import Spec, Src0, Src1, C0, C1, lower
    from concourse.dve_spec import _has_src1
    from concourse.dve_uop import DveOpSpec

    name = "TT_AFFINE_I16_ANT"
    for op in dvo.OPS:
        if op.name == name:
            _FUSED_OP = op
            return op

    spec = Spec(
        body=Src0 * Src1 * C0 + C1,
        reference=lambda in0, in1, s0, s1, imm2: (
            in0.astype(np.float32) * in1.astype(np.float32) * s0 + s1
        ),
    )
    row = dvo._CUSTOM_DVE_ROW_BASE + len(dvo.OPS)
    assert row < 0x20, "no free custom-DVE opcode rows"
    shas = {}
    for ver in ("v3", "v4"):
        tmp = DveOpSpec(
            name=name, opcode=row, uops=lower(spec, ver=ver),
            rd1_en=_has_src1(spec),
        )
        shas[ver] = tmp.sha(ver)
    op = dvo.DveOp(name, spec, subdim=False, uops_sha=shas)
    dvo.OPS.append(op)
    dvo.CUSTOM_DVE_SPECS[name] = spec
    dvo._SUB_OPCODE_FOR_NAME[name] = row
    _FUSED_OP = op
    return op


def _build(generic: bool, exp_mode: str):
    nc = bacc.Bacc("TRN2", target_bir_lowering=False, debug=False,
                   num_devices=ND)

    hT_d = nc.dram_tensor("hT", [F, N], bf16, kind="ExternalInput").ap()
    atp_d = nc.dram_tensor("atp", [N, L], bf16, kind="ExternalInput").ap()
    wqT_d = nc.dram_tensor("wqT", [F, DH], bf16, kind="ExternalInput").ap()
    wkT_d = nc.dram_tensor("wkT", [F, F], bf16, kind="ExternalInput").ap()
    wvT_d = nc.dram_tensor("wvT", [F, F], bf16, kind="ExternalInput").ap()
    woT_d = nc.dram_tensor("woT", [F, F], bf16, kind="ExternalInput").ap()
    w1T_d = nc.dram_tensor("w1T", [F, F2], bf16, kind="ExternalInput").ap()
    w2T_d = nc.dram_tensor("w2T", [F2, F], bf16, kind="ExternalInput").ap()
    h1T_d = nc.dram_tensor("h1T", [F, L], bf16, kind="ExternalInput").ap()
    if generic:
        vecs_d = nc.dram_tensor("vecs", [128, NVEC], f32,
                                kind="ExternalInput").ap()
        bvrow_d = nc.dram_tensor("bvrow", [1, F], f32,
                                 kind="ExternalInput").ap()
    out_d = nc.dram_tensor("out", [F, L], f32, kind="ExternalOutput").ap()

    # collective staging (DRAM only)
    dbg = os.environ.get("BASS_GT_DEBUG") == "1"
    ot_dram = nc.dram_tensor("ot_stage", [H * DH, L], bf16)
    ya_dram = nc.dram_tensor("ya_stage", [H * DH, L], bf16)
    if dbg:
        ot_dump = nc.dram_tensor("ot_dump", [H * DH, L], bf16,
                                 kind="ExternalOutput").ap()
        ya_dump = nc.dram_tensor("ya_dump", [H * DH, L], bf16,
                                 kind="ExternalOutput").ap()
        ps_dump = nc.dram_tensor("ps_dump", [128, 1024], f32,
                                 kind="ExternalOutput").ap()
        et_dump = nc.dram_tensor("et_dump", [128, 1024], i16,
                                 kind="ExternalOutput").ap()
        pso_dump = nc.dram_tensor("pso_dump", [128, 256], f32,
                                  kind="ExternalOutput").ap()
        r_dump = nc.dram_tensor("r_dump", [64, 256], f32,
                                kind="ExternalOutput").ap()
        v4_dump = nc.dram_tensor("v4_dump", [128, 64], fp16,
                                 kind="ExternalOutput").ap()
    st1_in = nc.dram_tensor("st1_in", [128, 4], f32)
    st1_out = nc.dram_tensor("st1_out", [128, 4], f32, addr_space="Shared")
    st2_in = nc.dram_tensor("st2_in", [128, 4], f32)
    st2_out = nc.dram_tensor("st2_out", [128, 4], f32, addr_space="Shared")

    groups = [list(range(ND))]
    use_fused = exp_mode == "custom"
    if use_fused:
        fused_op = _get_fused_op()

    with tile.TileContext(nc) as tc, ExitStack() as ctx:
        res = ctx.enter_context(tc.tile_pool(name="res", bufs=1))
        psp = ctx.enter_context(tc.tile_pool(name="psp", bufs=2, space="PSUM"))
        pso = ctx.enter_context(tc.tile_pool(name="pso", bufs=2, space="PSUM"))
        pj = ctx.enter_context(tc.tile_pool(name="pj", bufs=2, space="PSUM"))
        et_pool = ctx.enter_context(tc.tile_pool(name="etp", bufs=2))
        pt_pool = ctx.enter_context(tc.tile_pool(name="ptp", bufs=2))
        small = ctx.enter_context(tc.tile_pool(name="small", bufs=2))
        ffn = ctx.enter_context(tc.tile_pool(name="ffn", bufs=1))

        # ---- resident tensors ----
        ht = res.tile([128, 2, N], bf16)          # h^T: [f%128, f//128, n]
        at_sb = res.tile([128, 32, 512], bf16)    # A^T tiles, custom order
        kt = res.tile([128, 2, N], bf16)          # K^T
        qt4 = res.tile([128, N], bf16)            # Q^T slice, 4x replicated
        v4 = res.tile([128, 32, 8, 64], fp16)     # [V | ones] per (nt, sp)
        h1_sb = res.tile([128, 2, L], bf16)       # h^T[:, d-block] residual
        wq_sb = res.tile([128, 2, DH], bf16)
        wk_sb = res.tile([128, 2, F], bf16)
        wv_sb = res.tile([128, 2, F], bf16)
        wo_sb = res.tile([128, 2, F], bf16)
        w1_sb = res.tile([128, 2, F2], bf16)
        w2_sb = res.tile([128, 4, F], bf16)
        if generic:
            vecs = res.tile([128, NVEC], f32)
            nc.sync.dma_start(out=vecs, in_=vecs_d)
            bvb = res.tile([128, F], f32)
            nc.sync.dma_start(out=bvb, in_=bvrow_d.to_broadcast([128, F]))

        nc.sync.dma_start(out=wq_sb[:, 0, :], in_=wqT_d[0:128, :])
        nc.sync.dma_start(out=wq_sb[:, 1, :], in_=wqT_d[128:256, :])
        for gc in range(2):
            nc.sync.dma_start(out=wk_sb[:, gc, :],
                              in_=wkT_d[gc * 128:(gc + 1) * 128, :])
            nc.sync.dma_start(out=wv_sb[:, gc, :],
                              in_=wvT_d[gc * 128:(gc + 1) * 128, :])
        for ch in range(H):
            for gc in range(2):
                nc.sync.dma_start(
                    out=ht[:, gc, ch * 512:(ch + 1) * 512],
                    in_=hT_d[gc * 128:(gc + 1) * 128,
                             ch * 512:(ch + 1) * 512])
        for gc in range(2):
            nc.sync.dma_start(out=wo_sb[:, gc, :],
                              in_=woT_d[gc * 128:(gc + 1) * 128, :])
            nc.sync.dma_start(out=w1_sb[:, gc, :],
                              in_=w1T_d[gc * 128:(gc + 1) * 128, :])
            nc.sync.dma_start(out=h1_sb[:, gc, :],
                              in_=h1T_d[gc * 128:(gc + 1) * 128, :])
        for kc in range(4):
            nc.sync.dma_start(out=w2_sb[:, kc, :],
                              in_=w2T_d[kc * 128:(kc + 1) * 128, :])
        nc.vector.memset(v4[:, :, :, DH:2 * DH], 1.0)

        def emit_proj(hh):
            """Q/K/V projections for one head's slices."""
            cs = slice(hh * 512, (hh + 1) * 512)
            # Q^T chunk -> qt4[0:32, cs], then replicate to bands 1..3
            pq = pj.tile([128, 512], f32, tag="pj")
            for gc in range(2):
                nc.tensor.matmul(pq[0:DH, :], lhsT=wq_sb[:, gc, :],
                                 rhs=ht[:, gc, cs],
                                 start=(gc == 0), stop=(gc == 1))
            if generic:
                nc.vector.tensor_scalar_add(
                    qt4[0:DH, cs], pq[0:DH, :],
                    vecs[0:DH, VEC_BQ:VEC_BQ + 1])
            else:
                nc.scalar.copy(qt4[0:DH, cs], pq[0:DH, :])
            for band in range(1, 4):
                nc.sync.dma_start(out=qt4[band * DH:(band + 1) * DH, cs],
                                  in_=qt4[0:DH, cs])
            # K^T chunks
            for hf in range(2):
                pk = pj.tile([128, 512], f32, tag="pj")
                for gc in range(2):
                    nc.tensor.matmul(
                        pk, lhsT=wk_sb[:, gc, hf * 128:(hf + 1) * 128],
                        rhs=ht[:, gc, cs],
                        start=(gc == 0), stop=(gc == 1))
                if generic:
                    nc.vector.tensor_scalar_add(
                        kt[:, hf, cs], pk, vecs[:, VEC_BK + hf:VEC_BK + hf + 1])
                else:
                    nc.scalar.copy(kt[:, hf, cs], pk)
            # V natural tiles (with room for the ones block)
            for up in range(4):
                nt = hh * 4 + up
                pv = pj.tile([128, 512], f32, tag="pj")
                for gc in range(2):
                    nc.tensor.matmul(pv[:, 0:F],
                                     lhsT=ht[:, gc, nt * 128:(nt + 1) * 128],
                                     rhs=wv_sb[:, gc, :],
                                     start=(gc == 0), stop=(gc == 1))
                src = pv[:, 0:F].rearrange("p (s c) -> p s c", c=DH)
                if generic:
                    nc.vector.tensor_add(
                        v4[:, nt, :, 0:DH], src,
                        bvb.rearrange("p (s c) -> p s c", c=DH))
                else:
                    nc.scalar.copy(v4[:, nt, :, 0:DH], src)

        # head-0 projections first so attention starts early; A^T tile DMAs
        # stream in group order under head 0's compute.
        emit_proj(0)
        for hf in range(2):
            for up in range(4):
                for pair in range(2):
                    for bi in range(2):
                        idx = ((hf * 2 + pair) * 4 + up) * 2 + bi
                        s2 = hf * 4 + pair * 2 + bi
                        src = s2 * 4 + up
                        nc.sync.dma_start(
                            out=at_sb[:, idx, :],
                            in_=atp_d[src * 128:(src + 1) * 128, :])

        # ---- attention (projections for head hh+1 interleaved) ----
        for hh in range(H):
            pso_t = pso.tile([128, 256], f32, tag="pso")
            first = [True, True]          # per m1-half accumulation chain
            n_grp = 0
            for hf in range(2):
                for up in range(4):
                    for pair in range(2):
                        n_grp += 1
                        g2 = (hf * 2 + pair) * 4 + up
                        psp_t = psp.tile([128, 2, 512], f32, tag="psp")
                        for bi in range(2):
                            band = pair * 2 + bi
                            nc.tensor.matmul(
                                psp_t[:, bi, :],
                                lhsT=kt[band * DH:(band + 1) * DH, hf,
                                        hh * 512 + up * 128:
                                        hh * 512 + (up + 1) * 128],
                                rhs=qt4[band * DH:(band + 1) * DH,
                                        hh * 512:(hh + 1) * 512],
                                start=True, stop=True,
                                tile_position=(band * DH, 0))
                        at_ap = at_sb[:, g2 * 2:g2 * 2 + 2, :]
                        is_dbg_tile = dbg and hh == 0 and hf == 0 \
                            and up == 0 and pair == 0
                        if is_dbg_tile:
                            ps_scr = small.tile([128, 1024], f32,
                                                tag="psscr", bufs=1)
                            nc.scalar.copy(
                                ps_scr, psp_t.rearrange("p a b -> p (a b)"))
                            nc.sync.dma_start(out=ps_dump, in_=ps_scr)
                        if use_fused:
                            et_t = et_pool.tile([128, 2, 512], i16, tag="et")
                            nc.vector._custom_dve(
                                fused_op, out=et_t, in0=psp_t, in1=at_ap,
                                s0=A_E, s1=B_E)
                        else:
                            pt_t = pt_pool.tile([128, 2, 512], bf16, tag="pt")
                            nc.vector.tensor_mul(pt_t, psp_t, at_ap)
                            et_t = et_pool.tile([128, 2, 512], fp16, tag="et")
                            nc.scalar.activation(
                                et_t, pt_t, mybir.ActivationFunctionType.Exp,
                                scale=SCALE)
                        if is_dbg_tile:
                            nc.sync.dma_start(
                                out=et_dump,
                                in_=et_t.rearrange("p a b -> p (a b)"))
                            nc.sync.dma_start(out=v4_dump,
                                              in_=v4[:, 0, 0, :])
                        last_grp = n_grp == 16
                        for bi in range(2):
                            sp = hf * 4 + pair * 2 + bi
                            nt = hh * 4 + up
                            for mh in range(2):
                                rhs = et_t[:, bi, mh * 256:(mh + 1) * 256]
                                if use_fused:
                                    rhs = rhs.bitcast(fp16)
                                nc.tensor.matmul(
                                    pso_t[mh * 64:(mh + 1) * 64, :],
                                    lhsT=v4[:, nt, sp, :],
                                    rhs=rhs,
                                    start=first[mh],
                                    stop=(last_grp and bi == 1),
                                    tile_position=(0, mh * 64))
                                first[mh] = False
            # normalize: rows [mh*64+32, mh*64+64) hold the denominators
            if dbg and hh == 0:
                pso_scr = small.tile([128, 256], f32, tag="psoscr", bufs=1)
                nc.scalar.copy(pso_scr, pso_t)
                nc.sync.dma_start(out=pso_dump, in_=pso_scr)
            ot_sb = small.tile([DH, 512], bf16, tag="ot")
            for mh in range(2):
                den_sb = small.tile([DH, 256], f32, tag="den", bufs=2)
                nc.scalar.copy(den_sb,
                               pso_t[mh * 64 + DH:mh * 64 + 2 * DH, :])
                r = small.tile([DH, 256], f32, tag="r", bufs=2)
                nc.vector.reciprocal_approx_fast(r, den_sb)
                if dbg and hh == 0:
                    nc.sync.dma_start(
                        out=r_dump[mh * DH:(mh + 1) * DH, :], in_=r)
                nc.vector.tensor_mul(
                    ot_sb[:, mh * 256:(mh + 1) * 256],
                    pso_t[mh * 64:mh * 64 + DH, :], r)
            nc.sync.dma_start(out=ot_dram.ap()[hh * DH:(hh + 1) * DH, :],
                              in_=ot_sb)
            if dbg:
                nc.sync.dma_start(out=ot_dump[hh * DH:(hh + 1) * DH, :],
                                  in_=ot_sb)
            if hh + 1 < H:
                emit_proj(hh + 1)

        # ---- exchange to row-blocks ----
        nc.gpsimd.collective_compute(
            "AllToAll", mybir.AluOpType.bypass, replica_groups=groups,
            ins=[ot_dram.ap()], outs=[ya_dram.ap()])

        yt = ffn.tile([128, 2, L], bf16, tag="yt")
        for gc in range(2):
            nc.sync.dma_start(out=yt[:, gc, :],
                              in_=ya_dram.ap()[gc * 128:(gc + 1) * 128, :])
            if dbg:
                nc.sync.dma_start(out=ya_dump[gc * 128:(gc + 1) * 128, :],
                                  in_=yt[:, gc, :])

        # ---- Wo + residual -> x1 ; BN1 stats ----
        x1 = ffn.tile([128, 2, L], bf16, tag="x1")
        stat_in = ffn.tile([128, 4], f32, tag="stat")
        sq_scr = ffn.tile([128, L], bf16, tag="sq")
        for fo in range(2):
            py = pj.tile([128, 512], f32, tag="pj")
            for gc in range(2):
                nc.tensor.matmul(
                    py,
                    lhsT=wo_sb[:, gc, fo * 128:(fo + 1) * 128],
                    rhs=yt[:, gc, :],
                    start=(gc == 0), stop=(gc == 1))
            if generic:
                nc.vector.tensor_scalar_add(py, py,
                                            vecs[:, VEC_BO + fo:VEC_BO + fo + 1])
            nc.vector.tensor_add(x1[:, fo, :], py, h1_sb[:, fo, :])
            nc.vector.tensor_reduce(stat_in[:, fo:fo + 1], x1[:, fo, :],
                                    axis=mybir.AxisListType.X,
                                    op=mybir.AluOpType.add)
            nc.scalar.activation(sq_scr, x1[:, fo, :],
                                 mybir.ActivationFunctionType.Square,
                                 accum_out=stat_in[:, 2 + fo:3 + fo])
        nc.sync.dma_start(out=st1_in.ap(), in_=stat_in)
        nc.gpsimd.collective_compute(
            "AllReduce", mybir.AluOpType.add, replica_groups=groups,
            ins=[st1_in.ap()], outs=[st1_out.ap()])
        st1 = ffn.tile([128, 4], f32, tag="st1")
        nc.sync.dma_start(out=st1, in_=st1_out.ap())

        def bn_affine(st, vg, vbe):
            """per-half affine (a, b): out = x*a - b."""
            a_list, b_list = [], []
            for hfi in range(2):
                mu = small.tile([128, 1], f32, tag="mu", bufs=1)
                nc.vector.tensor_scalar_mul(mu, st[:, hfi:hfi + 1], 1.0 / N)
                ex2 = small.tile([128, 1], f32, tag="ex2", bufs=1)
                nc.vector.tensor_scalar_mul(ex2, st[:, 2 + hfi:3 + hfi],
                                            1.0 / N)
                var = small.tile([128, 1], f32, tag="var", bufs=1)
                nc.vector.tensor_mul(var, mu, mu)
                nc.vector.tensor_sub(var, ex2, var)
                nc.vector.tensor_scalar_add(var, var, EPS)
                sd = small.tile([128, 1], f32, tag="sd", bufs=1)
                nc.scalar.sqrt(sd, var)
                rv = small.tile([128, 1], f32, tag="rv", bufs=1)
                nc.vector.reciprocal(rv, sd)
                if generic:
                    a = small.tile([128, 1], f32, tag=f"a{hfi}", bufs=1)
                    nc.vector.tensor_mul(a, vecs[:, vg + hfi:vg + hfi + 1], rv)
                    b = small.tile([128, 1], f32, tag=f"b{hfi}", bufs=1)
                    nc.vector.tensor_mul(b, mu, a)
                    nc.vector.tensor_sub(b, b,
                                         vecs[:, vbe + hfi:vbe + hfi + 1])
                else:
                    a = rv
                    b = small.tile([128, 1], f32, tag=f"b{hfi}", bufs=1)
                    nc.vector.tensor_mul(b, mu, a)
                a_list.append(a)
                b_list.append(b)
            return a_list, b_list

        a1, b1 = bn_affine(st1, VEC_G1, VEC_BE1)
        x2 = ffn.tile([128, 2, L], bf16, tag="x2")
        for hfi in range(2):
            nc.vector.tensor_scalar(x2[:, hfi, :], x1[:, hfi, :],
                                    a1[hfi], b1[hfi],
                                    op0=mybir.AluOpType.mult,
                                    op1=mybir.AluOpType.subtract)

        # ---- FFN ----
        za = ffn.tile([128, 4, L], bf16, tag="za")
        for f2t in range(4):
            pz = pj.tile([128, 512], f32, tag="pj")
            for gc in range(2):
                nc.tensor.matmul(
                    pz,
                    lhsT=w1_sb[:, gc, f2t * 128:(f2t + 1) * 128],
                    rhs=x2[:, gc, :],
                    start=(gc == 0), stop=(gc == 1))
            if generic:
                nc.scalar.activation(za[:, f2t, :], pz,
                                     mybir.ActivationFunctionType.Relu,
                                     bias=vecs[:, VEC_C1 + f2t:VEC_C1 + f2t + 1])
            else:
                nc.scalar.activation(za[:, f2t, :], pz,
                                     mybir.ActivationFunctionType.Relu)

        x3 = ffn.tile([128, 2, L], bf16, tag="x3")
        stat2 = ffn.tile([128, 4], f32, tag="stat2")
        for fo in range(2):
            p2 = pj.tile([128, 512], f32, tag="pj")
            for kc in range(4):
                nc.tensor.matmul(
                    p2,
                    lhsT=w2_sb[:, kc, fo * 128:(fo + 1) * 128],
                    rhs=za[:, kc, :],
                    start=(kc == 0), stop=(kc == 3))
            if generic:
                nc.vector.tensor_scalar_add(p2, p2,
                                            vecs[:, VEC_C2 + fo:VEC_C2 + fo + 1])
            nc.vector.tensor_add(x3[:, fo, :], p2, x2[:, fo, :])
            nc.vector.tensor_reduce(stat2[:, fo:fo + 1], x3[:, fo, :],
                                    axis=mybir.AxisListType.X,
                                    op=mybir.AluOpType.add)
            nc.scalar.activation(sq_scr, x3[:, fo, :],
                                 mybir.ActivationFunctionType.Square,
                                 accum_out=stat2[:, 2 + fo:3 + fo])
        nc.sync.dma_start(out=st2_in.ap(), in_=stat2)
        nc.gpsimd.collective_compute(
            "AllReduce", mybir.AluOpType.add, replica_groups=groups,
            ins=[st2_in.ap()], outs=[st2_out.ap()])
        st2 = ffn.tile([128, 4], f32, tag="st2")
        nc.sync.dma_start(out=st2, in_=st2_out.ap())

        a2, b2 = bn_affine(st2, VEC_G2, VEC_BE2)
        for hfi in range(2):
            xo = small.tile([128, 512], f32, tag="xo", bufs=2)
            nc.vector.tensor_scalar(xo, x3[:, hfi, :], a2[hfi], b2[hfi],
                                    op0=mybir.AluOpType.mult,
                                    op1=mybir.AluOpType.subtract)
            nc.sync.dma_start(out=out_d[hfi * 128:(hfi + 1) * 128, :], in_=xo)

    nc.compile()
    return nc


def _get_nc(generic, exp_mode):
    key = (generic, exp_mode)
    if key not in _CACHE:
        _CACHE[key] = _build(generic, exp_mode)
    return _CACHE[key]


def kernel(A, h, Wq, bq, Wk, bk, Wv, bv, Wo, bo, W1, c1, W2, c2,
           g1, be1, g2, be2):
    import ml_dtypes
    nbf = ml_dtypes.bfloat16

    A = np.asarray(A, np.float32)
    h = np.asarray(h, np.float32)

    idx = np.arange(N)
    perm = (idx % L) * H + idx // L        # m~ -> m
    Ap = A[np.ix_(perm, perm)]
    ApT = np.ascontiguousarray(Ap.T).astype(nbf)   # [m~', m~]
    hT = np.ascontiguousarray(h.T)
    hTb = hT.astype(nbf)

    def tb(x):
        return np.ascontiguousarray(np.asarray(x, np.float32).T).astype(nbf)

    wqT = tb(Wq)
    wkT = tb(Wk)
    wvT = tb(Wv)
    woT = tb(Wo)
    w1T = tb(W1)
    w2T = tb(W2)

    generic = any(
        np.any(np.asarray(v))
        for v in (bq, bk, bv, bo, c1, c2, be1, be2)
    ) or np.any(np.asarray(g1) != 1) or np.any(np.asarray(g2) != 1)
    exp_mode = os.environ.get("BASS_GT_EXPMODE", "custom")
    nc = _get_nc(generic, exp_mode)

    def halves(v):
        return np.asarray(v, np.float32).reshape(2, 128).T  # [128, 2]

    in_maps = []
    for d in range(ND):
        m = {
            "hT": hTb,
            "atp": np.ascontiguousarray(ApT[:, d * L:(d + 1) * L]),
            "wqT": np.ascontiguousarray(wqT[:, d * DH:(d + 1) * DH]),
            "wkT": wkT, "wvT": wvT, "woT": woT, "w1T": w1T, "w2T": w2T,
            "h1T": np.ascontiguousarray(hTb[:, d * L:(d + 1) * L]),
        }
        if generic:
            vecs = np.zeros((128, NVEC), np.float32)
            vecs[0:DH, VEC_BQ] = np.asarray(bq, np.float32)[d * DH:(d + 1) * DH]
            vecs[:, VEC_BK:VEC_BK + 2] = halves(bk)
            vecs[:, VEC_BO:VEC_BO + 2] = halves(bo)
            vecs[:, VEC_C1:VEC_C1 + 4] = np.asarray(c1, np.float32).reshape(4, 128).T
            vecs[:, VEC_C2:VEC_C2 + 2] = halves(c2)
            vecs[:, VEC_G1:VEC_G1 + 2] = halves(g1)
            vecs[:, VEC_BE1:VEC_BE1 + 2] = halves(be1)
            vecs[:, VEC_G2:VEC_G2 + 2] = halves(g2)
            vecs[:, VEC_BE2:VEC_BE2 + 2] = halves(be2)
            m["vecs"] = vecs
            m["bvrow"] = np.asarray(bv, np.float32).reshape(1, F)
        in_maps.append(m)

    res = run_bass_kernel_spmd(nc, in_maps, core_ids=list(range(ND)))
    out = np.concatenate(
        [np.asarray(r["out"]).T for r in res.results], axis=0)
    return out.astype(np.float32)


if __name__ == "__main__":
    pass


# revision 35
# speedup vs baseline: 2.4908x; 1.0137x over previous
"""Trainium2 Bass kernel for GTLayer (graph-transformer layer), 8-core SPMD.

Math (matching the torch-style reference exactly):
  QH = h @ Wq.T + bq ; KH, VH likewise                          [N, F]
  per head hh (raw reshape): q_hh = QH[hh*512:(hh+1)*512].view(N, 32)
  t = q @ k.T * scale ; P = softmax(t * A, axis=-1) ; O = P @ v
  y = concat-heads @ Wo.T + bo
  x = BN1(y + h); out = BN2(x + relu(x@W1.T+c1)@W2.T+c2)

Distribution: a row permutation m~ = s*512+u  <->  m = u*8+s turns every
head-view block into natural-layout slices (see v1 notes).  Device d owns
score rows m~ in [d*512, (d+1)*512): it computes S^T tiles (partition =
key m~', free = query m~), applies the multiplicative adjacency mask and
the exponential in a single fused custom DVE op (Schraudolph exp2: the
int16 value a*x+b IS the fp16 bit pattern of exp(scale*x)), then
accumulates O^T plus softmax denominators via an augmented [V | 1]
matmul, col-packed 2x by query halves.  All matmuls run in bf16/fp16
(fp32 PSUM).  An AllToAll (bf16) re-shards head-blocks to row-blocks for
Wo/BN/FFN, which run transposed (features on partitions) so BatchNorm
stats are per-partition sums reduced with a tiny AllReduce.
"""

import math
import os
import sys

sys.path.insert(0, "/opt/trn_rl_repo")

from contextlib import ExitStack

import numpy as np

import concourse.bacc as bacc
import concourse.bass as bass
import concourse.tile as tile
from concourse import mybir
from concourse.bass_utils import run_bass_kernel_spmd

ND = 8          # devices
N = 4096        # nodes
F = 256         # hidden
H = 8           # heads
DH = 32         # head dim
L = N // ND     # 512 rows per device
F2 = 2 * F      # ffn hidden
SCALE = DH ** -0.5
EPS = 1e-5
f32 = mybir.dt.float32
bf16 = mybir.dt.bfloat16
fp16 = mybir.dt.float16
i16 = mybir.dt.int16

# Schraudolph constants targeting the fp16 bit pattern:
#   int16 z = A_E * (t*A) + B_E  ==  fp16-bits of ~exp(SCALE * t*A)
A_E = 1024.0 * math.log2(math.e) * SCALE
B_E = float(15 << 10)

# vecs packing for the generic (nonzero-bias) path [128, NVEC]
VEC_BQ = 0        # bq slice d        (32 rows used)
VEC_BK = 1        # bk halves         (2 cols)
VEC_BO = 3        # bo halves         (2 cols)
VEC_C1 = 5        # c1 quarters       (4 cols)
VEC_C2 = 9        # c2 halves         (2 cols)
VEC_G1 = 11       # g1 halves         (2)
VEC_BE1 = 13      # be1 halves        (2)
VEC_G2 = 15       # g2 halves         (2)
VEC_BE2 = 17      # be2 halves        (2)
NVEC = 19
# V bias (varies along the free axis) ships as a broadcast row instead.

_CACHE = {}
_FUSED_OP = None


def _get_fused_op():
    """Register (once) the fused mask-multiply + Schraudolph-exp DVE op:
    out = (in0 * in1) * s0 + s1, written as int16 (the fp16 bit pattern)."""
    global _FUSED_OP
    if _FUSED_OP is not None:
        return _FUSED_OP
    import concourse.dve_ops as dvo
    from concourse.dve_spec import Spec, Src0, Src1, C0, C1, lower
    from concourse.dve_spec import _has_src1
    from concourse.dve_uop import DveOpSpec

    name = "TT_AFFINE_I16_ANT"
    for op in dvo.OPS:
        if op.name == name:
            _FUSED_OP = op
            return op

    spec = Spec(
        body=Src0 * Src1 * C0 + C1,
        reference=lambda in0, in1, s0, s1, imm2: (
            in0.astype(np.float32) * in1.astype(np.float32) * s0 + s1
        ),
    )
    row = dvo._CUSTOM_DVE_ROW_BASE + len(dvo.OPS)
    assert row < 0x20, "no free custom-DVE opcode rows"
    shas = {}
    for ver in ("v3", "v4"):
        tmp = DveOpSpec(
            name=name, opcode=row, uops=lower(spec, ver=ver),
            rd1_en=_has_src1(spec),
        )
        shas[ver] = tmp.sha(ver)
    op = dvo.DveOp(name, spec, subdim=False, uops_sha=shas)
    dvo.OPS.append(op)
    dvo.CUSTOM_DVE_SPECS[name] = spec
    dvo._SUB_OPCODE_FOR_NAME[name] = row
    _FUSED_OP = op
    return op


def _build(generic: bool, exp_mode: str):
    nc = bacc.Bacc("TRN2", target_bir_lowering=False, debug=False,
                   num_devices=ND)

    hT_d = nc.dram_tensor("hT", [F, N], bf16, kind="ExternalInput").ap()
    atp_d = nc.dram_tensor("atp", [N, L], bf16, kind="ExternalInput").ap()
    # wqkv = [wq tiled 4x | wk | wv]  [F, 128+256+256]
    wqkv_d = nc.dram_tensor("wqkv", [F, 640], bf16, kind="ExternalInput").ap()
    # wtail = [wo | w1 | h1]  [F, 256+512+512]
    wtail_d = nc.dram_tensor("wtail", [F, 1280], bf16,
                             kind="ExternalInput").ap()
    w2T_d = nc.dram_tensor("w2T", [F2, F], bf16, kind="ExternalInput").ap()
    if generic:
        vecs_d = nc.dram_tensor("vecs", [128, NVEC], f32,
                                kind="ExternalInput").ap()
        bvrow_d = nc.dram_tensor("bvrow", [1, F], f32,
                                 kind="ExternalInput").ap()
    out_d = nc.dram_tensor("out", [F, L], f32, kind="ExternalOutput").ap()

    # collective staging (DRAM only)
    dbg = os.environ.get("BASS_GT_DEBUG") == "1"
    ot_dram = nc.dram_tensor("ot_stage", [H * DH, L], bf16)
    ya_dram = nc.dram_tensor("ya_stage", [H * DH, L], bf16)
    if dbg:
        ot_dump = nc.dram_tensor("ot_dump", [H * DH, L], bf16,
                                 kind="ExternalOutput").ap()
        ya_dump = nc.dram_tensor("ya_dump", [H * DH, L], bf16,
                                 kind="ExternalOutput").ap()
        ps_dump = nc.dram_tensor("ps_dump", [128, 1024], f32,
                                 kind="ExternalOutput").ap()
        et_dump = nc.dram_tensor("et_dump", [128, 1024], i16,
                                 kind="ExternalOutput").ap()
        pso_dump = nc.dram_tensor("pso_dump", [128, 256], f32,
                                  kind="ExternalOutput").ap()
        r_dump = nc.dram_tensor("r_dump", [64, 256], f32,
                                kind="ExternalOutput").ap()
        v4_dump = nc.dram_tensor("v4_dump", [128, 64], fp16,
                                 kind="ExternalOutput").ap()
    st1_in = nc.dram_tensor("st1_in", [128, 4], f32)
    st1_out = nc.dram_tensor("st1_out", [128, 4], f32, addr_space="Shared")
    st2_in = nc.dram_tensor("st2_in", [128, 4], f32)
    st2_out = nc.dram_tensor("st2_out", [128, 4], f32, addr_space="Shared")

    groups = [list(range(ND))]
    use_fused = exp_mode == "custom"
    if use_fused:
        fused_op = _get_fused_op()
    # which of the 16 per-head groups take the ACT-offload exp route
    n_act = int(os.environ.get("BASS_GT_NACT", "6"))
    ACT_GRPS = set(range(0, 16, max(1, 16 // max(1, n_act)))[:n_act]) \
        if n_act else set()

    with tile.TileContext(nc) as tc, ExitStack() as ctx:
        res = ctx.enter_context(tc.tile_pool(name="res", bufs=1))
        psp = ctx.enter_context(tc.tile_pool(name="psp", bufs=2, space="PSUM"))
        pso = ctx.enter_context(tc.tile_pool(name="pso", bufs=2, space="PSUM"))
        pj = ctx.enter_context(tc.tile_pool(name="pj", bufs=2, space="PSUM"))
        et_pool = ctx.enter_context(tc.tile_pool(name="etp", bufs=2))
        pt_pool = ctx.enter_context(tc.tile_pool(name="ptp", bufs=2))
        sc_pool = ctx.enter_context(tc.tile_pool(name="scp", bufs=2))
        small = ctx.enter_context(tc.tile_pool(name="small", bufs=2))
        ffn = ctx.enter_context(tc.tile_pool(name="ffn", bufs=1))

        # ---- resident tensors ----
        ht = res.tile([128, 2, N], bf16)          # h^T: [f%128, f//128, n]
        at_sb = res.tile([128, 32, 512], bf16)    # A^T tiles, natural order
        kt = res.tile([128, 2, N], bf16)          # K^T
        qt4 = res.tile([128, N], bf16)            # Q^T slice, 4x replicated
        v4 = res.tile([128, 32, 8, 64], fp16)     # [V | ones] per (nt, sp)
        wqkv_sb = res.tile([128, 2, 640], bf16)
        wtail_sb = res.tile([128, 2, 1280], bf16)
        w2_sb = res.tile([128, 4, F], bf16)
        wq_sb = wqkv_sb[:, :, 0:128]              # wq tiled 4x
        wk_sb = wqkv_sb[:, :, 128:384]
        wv_sb = wqkv_sb[:, :, 384:640]
        wo_sb = wtail_sb[:, :, 0:256]
        w1_sb = wtail_sb[:, :, 256:768]
        h1_sb = wtail_sb[:, :, 768:1280]
        if generic:
            vecs = res.tile([128, NVEC], f32)
            nc.sync.dma_start(out=vecs, in_=vecs_d)
            bvb = res.tile([128, F], f32)
            nc.sync.dma_start(out=bvb, in_=bvrow_d.to_broadcast([128, F]))

        for gc in range(2):
            nc.sync.dma_start(out=wqkv_sb[:, gc, :],
                              in_=wqkv_d[gc * 128:(gc + 1) * 128, :])
        # h^T: first head-chunk pair small for a fast start, rest bulk
        for gc in range(2):
            nc.sync.dma_start(out=ht[:, gc, 0:1024],
                              in_=hT_d[gc * 128:(gc + 1) * 128, 0:1024])
        nc.vector.memset(v4[:, :, :, DH:2 * DH], 1.0)

        def emit_proj(hh):
            """Q/K/V projections for one head's slices."""
            cs = slice(hh * 512, (hh + 1) * 512)
            # Q^T chunk, 4 replicated bands in one shot (wq pre-tiled 4x)
            pq = pj.tile([128, 512], f32, tag="pj")
            for gc in range(2):
                nc.tensor.matmul(pq, lhsT=wq_sb[:, gc, :],
                                 rhs=ht[:, gc, cs],
                                 start=(gc == 0), stop=(gc == 1))
            if generic:
                nc.vector.tensor_scalar_add(
                    qt4[:, cs], pq, vecs[:, VEC_BQ:VEC_BQ + 1])
            else:
                nc.scalar.copy(qt4[:, cs], pq)
            # K^T chunks
            for hf in range(2):
                pk = pj.tile([128, 512], f32, tag="pj")
                for gc in range(2):
                    nc.tensor.matmul(
                        pk, lhsT=wk_sb[:, gc, hf * 128:(hf + 1) * 128],
                        rhs=ht[:, gc, cs],
                        start=(gc == 0), stop=(gc == 1))
                if generic:
                    nc.vector.tensor_scalar_add(
                        kt[:, hf, cs], pk, vecs[:, VEC_BK + hf:VEC_BK + hf + 1])
                else:
                    nc.scalar.copy(kt[:, hf, cs], pk)
            # V natural tiles (with room for the ones block)
            for up in range(4):
                nt = hh * 4 + up
                pv = pj.tile([128, 512], f32, tag="pj")
                for gc in range(2):
                    nc.tensor.matmul(pv[:, 0:F],
                                     lhsT=ht[:, gc, nt * 128:(nt + 1) * 128],
                                     rhs=wv_sb[:, gc, :],
                                     start=(gc == 0), stop=(gc == 1))
                src = pv[:, 0:F].rearrange("p (s c) -> p s c", c=DH)
                if generic:
                    nc.vector.tensor_add(
                        v4[:, nt, :, 0:DH], src,
                        bvb.rearrange("p (s c) -> p s c", c=DH))
                else:
                    nc.scalar.copy(v4[:, nt, :, 0:DH], src)

        # head-0 projections first so attention starts early; A^T streams in
        # two bulk DMAs (natural row-chunk order: tile t = s2*4+up) and the
        # rest of h^T / tail weights follow.
        emit_proj(0)
        atp_v = atp_d.rearrange("(c p) j -> p c j", p=128)   # [128, 32, 512]
        nc.sync.dma_start(out=at_sb[:, 0:16, :], in_=atp_v[:, 0:16, :])
        for gc in range(2):
            nc.sync.dma_start(out=ht[:, gc, 1024:4096],
                              in_=hT_d[gc * 128:(gc + 1) * 128, 1024:4096])
        nc.sync.dma_start(out=at_sb[:, 16:32, :], in_=atp_v[:, 16:32, :])
        for gc in range(2):
            nc.sync.dma_start(out=wtail_sb[:, gc, :],
                              in_=wtail_d[gc * 128:(gc + 1) * 128, :])
        nc.sync.dma_start(
            out=w2_sb, in_=w2T_d.rearrange("(c p) j -> p c j", p=128))

        # ---- attention (projections for head hh+1 interleaved) ----
        for hh in range(H):
            pso_t = pso.tile([128, 256], f32, tag="pso")
            first = [True, True]          # per m1-half accumulation chain
            n_grp = 0
            at_v = at_sb.rearrange("p (s u) j -> p s u j", u=4)
            for hf in range(2):
                for up in range(4):
                    for pair in range(2):
                        n_grp += 1
                        psp_t = psp.tile([128, 2, 512], f32, tag="psp")
                        for bi in range(2):
                            band = pair * 2 + bi
                            nc.tensor.matmul(
                                psp_t[:, bi, :],
                                lhsT=kt[band * DH:(band + 1) * DH, hf,
                                        hh * 512 + up * 128:
                                        hh * 512 + (up + 1) * 128],
                                rhs=qt4[band * DH:(band + 1) * DH,
                                        hh * 512:(hh + 1) * 512],
                                start=True, stop=True,
                                tile_position=(band * DH, 0))
                        s2a = hf * 4 + pair * 2
                        at_ap = at_v[:, s2a:s2a + 2, up, :]
                        is_dbg_tile = dbg and hh == 0 and hf == 0 \
                            and up == 0 and pair == 0
                        if is_dbg_tile:
                            ps_scr = small.tile([128, 1024], f32,
                                                tag="psscr", bufs=1)
                            nc.scalar.copy(
                                ps_scr, psp_t.rearrange("p a b -> p (a b)"))
                            nc.sync.dma_start(out=ps_dump, in_=ps_scr)
                        act_route = use_fused and (n_grp - 1) % 16 in ACT_GRPS
                        if act_route:
                            # offload: ACT evacuates scores to bf16, DVE runs
                            # the mask-multiply in 2x mode, ACT exponentiates
                            sc_t = sc_pool.tile([128, 2, 512], bf16, tag="sc")
                            nc.scalar.copy(sc_t, psp_t)
                            pt_t = pt_pool.tile([128, 2, 512], bf16, tag="pt")
                            nc.vector.tensor_mul(pt_t, sc_t, at_ap)
                            et_t = et_pool.tile([128, 2, 512], fp16, tag="et")
                            nc.scalar.activation(
                                et_t, pt_t, mybir.ActivationFunctionType.Exp,
                                scale=SCALE)
                            et_mm = et_t
                        elif use_fused:
                            et_t = et_pool.tile([128, 2, 512], i16, tag="et")
                            nc.vector._custom_dve(
                                fused_op, out=et_t, in0=psp_t, in1=at_ap,
                                s0=A_E, s1=B_E)
                            et_mm = None
                        else:
                            pt_t = pt_pool.tile([128, 2, 512], bf16, tag="pt")
                            nc.vector.tensor_mul(pt_t, psp_t, at_ap)
                            et_t = et_pool.tile([128, 2, 512], fp16, tag="et")
                            nc.scalar.activation(
                                et_t, pt_t, mybir.ActivationFunctionType.Exp,
                                scale=SCALE)
                            et_mm = et_t
                        if is_dbg_tile:
                            nc.sync.dma_start(
                                out=et_dump,
                                in_=et_t.rearrange("p a b -> p (a b)"))
                            nc.sync.dma_start(out=v4_dump,
                                              in_=v4[:, 0, 0, :])
                        last_grp = n_grp == 16
                        for bi in range(2):
                            sp = hf * 4 + pair * 2 + bi
                            nt = hh * 4 + up
                            for mh in range(2):
                                rhs = et_t[:, bi, mh * 256:(mh + 1) * 256]
                                if et_mm is None:
                                    rhs = rhs.bitcast(fp16)
                                nc.tensor.matmul(
                                    pso_t[mh * 64:(mh + 1) * 64, :],
                                    lhsT=v4[:, nt, sp, :],
                                    rhs=rhs,
                                    start=first[mh],
                                    stop=(last_grp and bi == 1),
                                    tile_position=(0, mh * 64))
                                first[mh] = False
            # normalize: rows [mh*64+32, mh*64+64) hold the denominators
            if dbg and hh == 0:
                pso_scr = small.tile([128, 256], f32, tag="psoscr", bufs=1)
                nc.scalar.copy(pso_scr, pso_t)
                nc.sync.dma_start(out=pso_dump, in_=pso_scr)
            ot_sb = small.tile([DH, 512], bf16, tag="ot")
            for mh in range(2):
                den_sb = small.tile([DH, 256], f32, tag="den", bufs=2)
                nc.scalar.copy(den_sb,
                               pso_t[mh * 64 + DH:mh * 64 + 2 * DH, :])
                r = small.tile([DH, 256], f32, tag="r", bufs=2)
                nc.vector.reciprocal_approx_fast(r, den_sb)
                if dbg and hh == 0:
                    nc.sync.dma_start(
                        out=r_dump[mh * DH:(mh + 1) * DH, :], in_=r)
                nc.vector.tensor_mul(
                    ot_sb[:, mh * 256:(mh + 1) * 256],
                    pso_t[mh * 64:mh * 64 + DH, :], r)
            nc.sync.dma_start(out=ot_dram.ap()[hh * DH:(hh + 1) * DH, :],
                              in_=ot_sb)
            if dbg:
                nc.sync.dma_start(out=ot_dump[hh * DH:(hh + 1) * DH, :],
                                  in_=ot_sb)
            if hh + 1 < H:
                emit_proj(hh + 1)

        # ---- exchange to row-blocks ----
        nc.gpsimd.collective_compute(
            "AllToAll", mybir.AluOpType.bypass, replica_groups=groups,
            ins=[ot_dram.ap()], outs=[ya_dram.ap()])

        yt = ffn.tile([128, 2, L], bf16, tag="yt")
        for gc in range(2):
            nc.sync.dma_start(out=yt[:, gc, :],
                              in_=ya_dram.ap()[gc * 128:(gc + 1) * 128, :])
            if dbg:
                nc.sync.dma_start(out=ya_dump[gc * 128:(gc + 1) * 128, :],
                                  in_=yt[:, gc, :])

        # ---- Wo + residual -> x1 ; BN1 stats ----
        x1 = ffn.tile([128, 2, L], bf16, tag="x1")
        stat_in = ffn.tile([128, 4], f32, tag="stat")
        sq_scr = ffn.tile([128, L], bf16, tag="sq")
        for fo in range(2):
            py = pj.tile([128, 512], f32, tag="pj")
            for gc in range(2):
                nc.tensor.matmul(
                    py,
                    lhsT=wo_sb[:, gc, fo * 128:(fo + 1) * 128],
                    rhs=yt[:, gc, :],
                    start=(gc == 0), stop=(gc == 1))
            if generic:
                nc.vector.tensor_scalar_add(py, py,
                                            vecs[:, VEC_BO + fo:VEC_BO + fo + 1])
            nc.vector.tensor_add(x1[:, fo, :], py, h1_sb[:, fo, :])
            nc.vector.tensor_reduce(stat_in[:, fo:fo + 1], x1[:, fo, :],
                                    axis=mybir.AxisListType.X,
                                    op=mybir.AluOpType.add)
            nc.scalar.activation(sq_scr, x1[:, fo, :],
                                 mybir.ActivationFunctionType.Square,
                                 accum_out=stat_in[:, 2 + fo:3 + fo])
        nc.sync.dma_start(out=st1_in.ap(), in_=stat_in)
        nc.gpsimd.collective_compute(
            "AllReduce", mybir.AluOpType.add, replica_groups=groups,
            ins=[st1_in.ap()], outs=[st1_out.ap()])
        st1 = ffn.tile([128, 4], f32, tag="st1")
        nc.sync.dma_start(out=st1, in_=st1_out.ap())

        def bn_affine(st, vg, vbe):
            """per-half affine (a, b): out = x*a - b."""
            a_list, b_list = [], []
            for hfi in range(2):
                mu = small.tile([128, 1], f32, tag="mu", bufs=1)
                nc.vector.tensor_scalar_mul(mu, st[:, hfi:hfi + 1], 1.0 / N)
                ex2 = small.tile([128, 1], f32, tag="ex2", bufs=1)
                nc.vector.tensor_scalar_mul(ex2, st[:, 2 + hfi:3 + hfi],
                                            1.0 / N)
                var = small.tile([128, 1], f32, tag="var", bufs=1)
                nc.vector.tensor_mul(var, mu, mu)
                nc.vector.tensor_sub(var, ex2, var)
                nc.vector.tensor_scalar_add(var, var, EPS)
                sd = small.tile([128, 1], f32, tag="sd", bufs=1)
                nc.scalar.sqrt(sd, var)
                rv = small.tile([128, 1], f32, tag="rv", bufs=1)
                nc.vector.reciprocal(rv, sd)
                if generic:
                    a = small.tile([128, 1], f32, tag=f"a{hfi}", bufs=1)
                    nc.vector.tensor_mul(a, vecs[:, vg + hfi:vg + hfi + 1], rv)
                    b = small.tile([128, 1], f32, tag=f"b{hfi}", bufs=1)
                    nc.vector.tensor_mul(b, mu, a)
                    nc.vector.tensor_sub(b, b,
                                         vecs[:, vbe + hfi:vbe + hfi + 1])
                else:
                    a = rv
                    b = small.tile([128, 1], f32, tag=f"b{hfi}", bufs=1)
                    nc.vector.tensor_mul(b, mu, a)
                a_list.append(a)
                b_list.append(b)
            return a_list, b_list

        a1, b1 = bn_affine(st1, VEC_G1, VEC_BE1)
        x2 = ffn.tile([128, 2, L], bf16, tag="x2")
        for hfi in range(2):
            nc.vector.tensor_scalar(x2[:, hfi, :], x1[:, hfi, :],
                                    a1[hfi], b1[hfi],
                                    op0=mybir.AluOpType.mult,
                                    op1=mybir.AluOpType.subtract)

        # ---- FFN ----
        za = ffn.tile([128, 4, L], bf16, tag="za")
        for f2t in range(4):
            pz = pj.tile([128, 512], f32, tag="pj")
            for gc in range(2):
                nc.tensor.matmul(
                    pz,
                    lhsT=w1_sb[:, gc, f2t * 128:(f2t + 1) * 128],
                    rhs=x2[:, gc, :],
                    start=(gc == 0), stop=(gc == 1))
            if generic:
                nc.scalar.activation(za[:, f2t, :], pz,
                                     mybir.ActivationFunctionType.Relu,
                                     bias=vecs[:, VEC_C1 + f2t:VEC_C1 + f2t + 1])
            else:
                nc.scalar.activation(za[:, f2t, :], pz,
                                     mybir.ActivationFunctionType.Relu)

        x3 = ffn.tile([128, 2, L], bf16, tag="x3")
        stat2 = ffn.tile([128, 4], f32, tag="stat2")
        for fo in range(2):
            p2 = pj.tile([128, 512], f32, tag="pj")
            for kc in range(4):
                nc.tensor.matmul(
                    p2,
                    lhsT=w2_sb[:, kc, fo * 128:(fo + 1) * 128],
                    rhs=za[:, kc, :],
                    start=(kc == 0), stop=(kc == 3))
            if generic:
                nc.vector.tensor_scalar_add(p2, p2,
                                            vecs[:, VEC_C2 + fo:VEC_C2 + fo + 1])
            nc.vector.tensor_add(x3[:, fo, :], p2, x2[:, fo, :])
            nc.vector.tensor_reduce(stat2[:, fo:fo + 1], x3[:, fo, :],
                                    axis=mybir.AxisListType.X,
                                    op=mybir.AluOpType.add)
            nc.scalar.activation(sq_scr, x3[:, fo, :],
                                 mybir.ActivationFunctionType.Square,
                                 accum_out=stat2[:, 2 + fo:3 + fo])
        nc.sync.dma_start(out=st2_in.ap(), in_=stat2)
        nc.gpsimd.collective_compute(
            "AllReduce", mybir.AluOpType.add, replica_groups=groups,
            ins=[st2_in.ap()], outs=[st2_out.ap()])
        st2 = ffn.tile([128, 4], f32, tag="st2")
        nc.sync.dma_start(out=st2, in_=st2_out.ap())

        a2, b2 = bn_affine(st2, VEC_G2, VEC_BE2)
        for hfi in range(2):
            xo = small.tile([128, 512], f32, tag="xo", bufs=2)
            nc.vector.tensor_scalar(xo, x3[:, hfi, :], a2[hfi], b2[hfi],
                                    op0=mybir.AluOpType.mult,
                                    op1=mybir.AluOpType.subtract)
            nc.sync.dma_start(out=out_d[hfi * 128:(hfi + 1) * 128, :], in_=xo)

    nc.compile()
    return nc


def _get_nc(generic, exp_mode):
    key = (generic, exp_mode)
    if key not in _CACHE:
        _CACHE[key] = _build(generic, exp_mode)
    return _CACHE[key]


def kernel(A, h, Wq, bq, Wk, bk, Wv, bv, Wo, bo, W1, c1, W2, c2,
           g1, be1, g2, be2):
    import ml_dtypes
    nbf = ml_dtypes.bfloat16

    A = np.asarray(A, np.float32)
    h = np.asarray(h, np.float32)

    idx = np.arange(N)
    perm = (idx % L) * H + idx // L        # m~ -> m
    Ap = A[np.ix_(perm, perm)]
    ApT = np.ascontiguousarray(Ap.T).astype(nbf)   # [m~', m~]
    hT = np.ascontiguousarray(h.T)
    hTb = hT.astype(nbf)

    def tb(x):
        return np.ascontiguousarray(np.asarray(x, np.float32).T).astype(nbf)

    wqT = tb(Wq)
    wkT = tb(Wk)
    wvT = tb(Wv)
    woT = tb(Wo)
    w1T = tb(W1)
    w2T = tb(W2)
    wkv = np.concatenate([wkT, wvT], axis=1)        # [256, 512]

    generic = any(
        np.any(np.asarray(v))
        for v in (bq, bk, bv, bo, c1, c2, be1, be2)
    ) or np.any(np.asarray(g1) != 1) or np.any(np.asarray(g2) != 1)
    exp_mode = os.environ.get("BASS_GT_EXPMODE", "custom")
    nc = _get_nc(generic, exp_mode)

    def halves(v):
        return np.asarray(v, np.float32).reshape(2, 128).T  # [128, 2]

    in_maps = []
    for d in range(ND):
        wq4 = np.tile(wqT[:, d * DH:(d + 1) * DH], (1, 4))   # [256, 128]
        wqkv = np.ascontiguousarray(
            np.concatenate([wq4, wkv], axis=1))              # [256, 640]
        wtail = np.ascontiguousarray(np.concatenate(
            [woT, w1T, hTb[:, d * L:(d + 1) * L]], axis=1))  # [256, 1280]
        m = {
            "hT": hTb,
            "atp": np.ascontiguousarray(ApT[:, d * L:(d + 1) * L]),
            "wqkv": wqkv, "wtail": wtail, "w2T": w2T,
        }
        if generic:
            vecs = np.zeros((128, NVEC), np.float32)
            vecs[:, VEC_BQ] = np.tile(
                np.asarray(bq, np.float32)[d * DH:(d + 1) * DH], 4)
            vecs[:, VEC_BK:VEC_BK + 2] = halves(bk)
            vecs[:, VEC_BO:VEC_BO + 2] = halves(bo)
            vecs[:, VEC_C1:VEC_C1 + 4] = np.asarray(c1, np.float32).reshape(4, 128).T
            vecs[:, VEC_C2:VEC_C2 + 2] = halves(c2)
            vecs[:, VEC_G1:VEC_G1 + 2] = halves(g1)
            vecs[:, VEC_BE1:VEC_BE1 + 2] = halves(be1)
            vecs[:, VEC_G2:VEC_G2 + 2] = halves(g2)
            vecs[:, VEC_BE2:VEC_BE2 + 2] = halves(be2)
            m["vecs"] = vecs
            m["bvrow"] = np.asarray(bv, np.float32).reshape(1, F)
        in_maps.append(m)

    res = run_bass_kernel_spmd(nc, in_maps, core_ids=list(range(ND)))
    out = np.concatenate(
        [np.asarray(r["out"]).T for r in res.results], axis=0)
    return out.astype(np.float32)


if __name__ == "__main__":
    pass
